# revision 20
# baseline (speedup 1.0000x reference)
"""Trainium2 Bass kernel for nn_AdvancedWebOfThoughts (gnn_message_passing).

Strategy:
  - Pure data parallel over 8 NeuronCores (1024 batch rows each), 8 tiles of
    128 rows per core.
  - Batch-major activations [128 batch partitions, features]; every weight
    matmul runs with the activation (feature-major, PE-transposed) as the
    stationary operand and the bf16 weight matrix as the moving operand, so
    matmul outputs land batch-major in PSUM.
  - LayerNorm: sum(z) rides the matmul as an extra weight column (w-bar);
    sum(z^2) comes from a ScalarE Square pass with accum_out; normalize
    (+ optional gelu) is one fused ScalarE activation with per-partition
    scale/bias.
  - Message passing attention: explicit per-(node, kv-node) K/V projections
    on the TensorEngine; scores / softmax / mix are batched DVE
    multiply + grouped-reduce ops in batch-major layout.

kernel() is self-contained: shapes hardcoded, all weight preprocessing is
host-side numpy.
"""

import sys

sys.path.insert(0, "/opt/trn_rl_repo")

import numpy as np
import ml_dtypes

import concourse.bass as bass  # noqa: F401
import concourse.bacc as bacc
import concourse.tile as tile
from concourse import mybir
from concourse.masks import make_identity
from concourse import bass_utils

BF16 = ml_dtypes.bfloat16
F32 = np.float32

D, H, NN, L, HEADS, T_, B = 768, 256, 8, 4, 4, 32, 8192
DH = H // HEADS
TASK = 2
EPS = 1e-5
NCORES = 8
BC = B // NCORES
PT = 128
SCALE = 1.0 / np.sqrt(DH)

AF = mybir.ActivationFunctionType
ALU = mybir.AluOpType
dt = mybir.dt


# ----------------------------------------------------------------------------
# Host-side weight prep
# ----------------------------------------------------------------------------

def _bf(a):
    return np.ascontiguousarray(np.asarray(a, F32)).astype(BF16)


def _pack_rhs(w, wbar=False):
    """w: [out_features, in_features] fp32 (torch Linear layout) ->
    (packed [128, nchunk*neff] bf16, nchunk, neff) in moving-operand layout."""
    wt = np.asarray(w, F32).T  # [K, N]
    K, N = wt.shape
    assert K % 128 == 0, (K, N)
    wt_bf = wt.astype(BF16)
    if wbar:
        col = wt_bf.astype(F32).sum(axis=1, keepdims=True)
        wt_bf = np.concatenate([wt_bf, col.astype(BF16)], axis=1)
        N += 1
    chunks = [wt_bf[c * 128:(c + 1) * 128] for c in range(K // 128)]
    return (np.ascontiguousarray(np.concatenate(chunks, axis=1)), K // 128, N)


def _bias_row(b, wbar=False):
    b = np.asarray(b, F32).reshape(1, -1)
    if wbar:
        b = np.concatenate([b, b.sum(axis=1, keepdims=True)], axis=1)
    return np.ascontiguousarray(b.astype(BF16))


def _repl(v):
    v = np.asarray(v, F32).reshape(1, -1)
    return np.ascontiguousarray(np.tile(v.astype(BF16), (128, 1)))


def _prep(params):
    p = {k: np.asarray(v, F32) for k, v in params.items()}
    a = {}
    a['we1'] = _pack_rhs(p['W_e1'], wbar=True)
    a['be1'] = _bias_row(p['b_e1'], wbar=True)
    a['ge1'] = (p['g_e1'], p['be_e1'])
    a['we2'] = _pack_rhs(p['W_e2'], wbar=True)
    a['be2'] = _bias_row(p['b_e2'], wbar=True)
    a['ge2'] = (p['g_e2'], p['be_e2'])

    te = p['task_emb'][TASK]
    for n in range(NN):
        w1 = p['Wn1'][n]
        b1 = p['bn1'][n] + w1[:, H:] @ te
        a[f'wn1_{n}'] = _pack_rhs(w1[:, :H], wbar=True)
        a[f'bn1_{n}'] = _bias_row(b1, wbar=True)
        a[f'gn1_{n}'] = (p['gn1'][n], p['bln1'][n])
        a[f'wn2_{n}'] = _pack_rhs(p['Wn2'][n], wbar=True)
        a[f'wn_{n}'] = (np.ascontiguousarray(np.concatenate(
            [a[f'wn1_{n}'][0], a[f'wn2_{n}'][0]], axis=1)), 0, 0)
        a[f'bn2_{n}'] = _bias_row(p['bn2'][n], wbar=True)
        a[f'gn2_{n}'] = (p['gn2'][n], p['bln2'][n])

        wq_p = _pack_rhs(p['Wea_in'][n][:H])
        wk_p = _pack_rhs(p['Wea_in'][n][H:2 * H])
        wv_p = _pack_rhs(p['Wea_in'][n][2 * H:])
        wo_p = _pack_rhs(p['Wea_out'][n])
        a[f'wq_{n}'] = wq_p
        a[f'wk_{n}'] = wk_p
        a[f'wv_{n}'] = wv_p
        a[f'wqk_{n}'] = (np.ascontiguousarray(
            np.concatenate([wq_p[0], wk_p[0]], axis=1)), 2, 256)
        a[f'wvo_{n}'] = (np.ascontiguousarray(
            np.concatenate([wv_p[0], wo_p[0]], axis=1)), 2, 256)
        a[f'bq_{n}'] = _bias_row(p['bea_in'][n][:H])
        a[f'bv_{n}'] = _bias_row(p['bea_in'][n][2 * H:])
        a[f'wo_{n}'] = _pack_rhs(p['Wea_out'][n])
        a[f'bo_{n}'] = _bias_row(p['bea_out'][n])

        a[f'wg1_{n}'] = _pack_rhs(p['Wg1'][n], wbar=True)
        a[f'bg1_{n}'] = _bias_row(p['bg1'][n], wbar=True)
        a[f'gg1_{n}'] = (p['gg1'][n], p['bgl1'][n])
        a[f'wg2_{n}'] = _repl(p['Wg2'][n][0])
    a['bg2'] = np.ascontiguousarray(
        np.tile(p['bg2'].reshape(1, NN).astype(F32), (128, 1)))

    a['wgq'] = _pack_rhs(p['Wga_in'][:H])
    a['bgq'] = _bias_row(p['bga_in'][:H])
    a['wgk'] = _pack_rhs(p['Wga_in'][H:2 * H])
    a['wgv'] = _pack_rhs(p['Wga_in'][2 * H:])
    a['bgv'] = _bias_row(p['bga_in'][2 * H:])
    a['wgo'] = _pack_rhs(p['Wga_out'], wbar=True)
    a['bgo'] = _bias_row(p['bga_out'], wbar=True)

    for l in range(L):
        wv_l = p['rs_Win'][l][2 * H:]
        bv_l = p['rs_bin'][l][2 * H:]
        w_vo = p['rs_Wout'][l] @ wv_l
        b_vo = p['rs_Wout'][l] @ bv_l + p['rs_bout'][l]
        a[f'rs_wvo_{l}'] = _pack_rhs(w_vo)
        a[f'rs_bvo_{l}'] = _bias_row(b_vo)
        a[f'rs_wf1_{l}'] = _pack_rhs(p['rs_Wf1'][l])
        a[f'rs_bf1_{l}'] = _bias_row(p['rs_bf1'][l])
        a[f'rs_wf2_{l}'] = _pack_rhs(p['rs_Wf2'][l])
        a[f'rs_bf2_{l}'] = _bias_row(p['rs_bf2'][l])
        a[f'rs_{l}'] = (np.ascontiguousarray(np.concatenate(
            [a[f'rs_wvo_{l}'][0], a[f'rs_wf1_{l}'][0], a[f'rs_wf2_{l}'][0]],
            axis=1)), 0, 0)
        a[f'rs_g1_{l}'] = (p['rs_g1'][l], p['rs_b1'][l])
        a[f'rs_g2_{l}'] = (p['rs_g2'][l], p['rs_b2'][l])

    a['wo1'] = _pack_rhs(p['Wo1'], wbar=True)
    a['bo1'] = _bias_row(p['bo1'], wbar=True)
    a['go1'] = (p['go1'], p['bol1'])
    a['wo2'] = _pack_rhs(p['Wo2'])          # no LN after Wo2
    a['bo2'] = _bias_row(p['bo2'])
    a['wf1'] = _pack_rhs(p['Wf1'], wbar=True)
    a['bf1'] = _bias_row(p['bf1'], wbar=True)
    a['gf1'] = (p['gf1'], p['bfl1'])
    a['wf2'] = _repl(p['Wf2'][0])
    a['bf2'] = float(np.asarray(p['bf2']).reshape(-1)[0])
    return a


def _affine_identity(gb):
    g, b = gb
    return np.allclose(g, 1.0) and np.allclose(b, 0.0)


def _zero(row):
    return not np.any(np.asarray(row, F32))


# ----------------------------------------------------------------------------
# Emitter
# ----------------------------------------------------------------------------

class Emitter:
    def __init__(self, nc, tc, ctx, arrays):
        self.nc = nc
        self.tc = tc
        self.arrays = arrays
        self.dram = {}
        self.wsb = {}

        self.p_w = ctx.enter_context(tc.tile_pool(name="wts", bufs=1))
        self.p_ws = ctx.enter_context(tc.tile_pool(name="wstream", bufs=2))
        self.p_const = ctx.enter_context(tc.tile_pool(name="const", bufs=1))
        self.p_act = ctx.enter_context(tc.tile_pool(name="act", bufs=2))
        self.p_act1 = ctx.enter_context(tc.tile_pool(name="act1", bufs=1))
        self.p_feat = ctx.enter_context(tc.tile_pool(name="feat", bufs=1))
        self.p_big = ctx.enter_context(tc.tile_pool(name="big", bufs=2))
        self.p_stat = ctx.enter_context(tc.tile_pool(name="stat", bufs=3))
        self.p_ps = ctx.enter_context(
            tc.tile_pool(name="ps", bufs=3, space="PSUM"))
        self.p_ps_tr = ctx.enter_context(
            tc.tile_pool(name="pstr", bufs=2, space="PSUM"))

        self.ident = self.p_const.tile([128, 128], dt.bfloat16, tag="ident")
        make_identity(nc, self.ident)
        self.ones_row = self.p_const.tile([1, 128], dt.bfloat16, tag="ones")
        nc.vector.memset(self.ones_row, 1.0)
        self.eps = self.p_const.tile([128, 1], dt.float32, tag="eps")
        nc.vector.memset(self.eps, EPS)
        self.p_scr = ctx.enter_context(tc.tile_pool(name="scr", bufs=3))

    def add_input(self, name, arr):
        h = self.nc.dram_tensor(name, list(arr.shape), dt.from_np(arr.dtype),
                                kind="ExternalInput")
        ap = h.ap()
        self.dram[name] = ap
        return ap

    def stage_weight(self, name):
        if name in self.wsb:
            return self.wsb[name]
        arr = self.arrays[name]
        if isinstance(arr, tuple):
            arr = arr[0]
        d = self.add_input("w_" + name, arr)
        t = self.p_w.tile(list(arr.shape), dt.from_np(arr.dtype), tag="w_" + name)
        self.nc.sync.dma_start(out=t, in_=d)
        self.wsb[name] = t
        return t

    def stream_weight(self, name, tag):
        """Per-tile staged weight (re-DMA'd each use; slot shared via tag)."""
        arr = self.arrays[name]
        if isinstance(arr, tuple):
            arr = arr[0]
        if ("w_" + name) not in self.dram:
            self.add_input("w_" + name, arr)
        d = self.dram["w_" + name]
        t = self.p_ws.tile(list(arr.shape), dt.from_np(arr.dtype), tag=tag)
        self.nc.sync.dma_start(out=t, in_=d)
        return t

    def mm(self, psum, actf, wname, bias=None, stream_tag=None, wt=None):
        nc = self.nc
        _, nchunk, neff = self.arrays[wname]
        if wt is None:
            if stream_tag is not None:
                wt = self.stream_weight(wname, stream_tag)
            else:
                wt = self.stage_weight(wname)
        segs = []
        off = 0
        while off < neff:
            seg = min(512, neff - off)
            segs.append((off, seg))
            off += seg
        have_bias = bias is not None and not _zero(self.arrays[bias])
        bt = self.stream_weight(bias, "s_brow") if have_bias else None
        for (off, seg) in segs:
            first = True
            if have_bias:
                nc.tensor.matmul(psum[:, off:off + seg], lhsT=self.ones_row,
                                 rhs=bt[:, off:off + seg], start=True,
                                 stop=False)
                first = False
            for c in range(nchunk):
                nc.tensor.matmul(psum[:, off:off + seg],
                                 lhsT=actf[:, c * 128:(c + 1) * 128],
                                 rhs=wt[:, c * neff + off:c * neff + off + seg],
                                 start=first, stop=(c == nchunk - 1))
                first = False

    def to_f(self, src, tag, pool=None):
        nc = self.nc
        W = src.shape[-1]
        f = (pool or self.p_act).tile([128, W], dt.bfloat16, tag=tag)
        nch = W // 128
        c = 0
        while c < nch:
            g = min(4, nch - c)
            tp = self.p_ps_tr.tile([128, 512], dt.bfloat16, tag="trp")
            for i in range(g):
                nc.tensor.transpose(tp[:, i * 128:(i + 1) * 128],
                                    src[:, (c + i) * 128:(c + i + 1) * 128],
                                    self.ident)
            nc.vector.tensor_copy(out=f[:, c * 128:(c + g) * 128],
                                  in_=tp[:, :g * 128])
            c += g
        return f

    def sqscr(self, W):
        return self.p_scr.tile([128, W], dt.bfloat16, tag="sq")

    def ln_stats(self, z_ps, W, sums, ssq):
        nc = self.nc
        nc.vector.tensor_copy(out=sums, in_=z_ps[:, W:W + 1])
        nc.scalar.activation(self.sqscr(W), z_ps[:, :W], AF.Square,
                             accum_out=ssq)

    def ln_finalize(self, sums, ssq, W, n):
        nc = self.nc
        mu = self.p_stat.tile([128, n], dt.float32, tag="mu")
        nc.vector.tensor_scalar(out=mu, in0=sums, scalar1=1.0 / W, scalar2=None,
                                op0=ALU.mult)
        musq = self.p_stat.tile([128, n], dt.float32, tag="musq")
        nc.vector.tensor_tensor(out=musq, in0=mu, in1=mu, op=ALU.mult)
        var = self.p_stat.tile([128, n], dt.float32, tag="var")
        nc.vector.scalar_tensor_tensor(out=var, in0=ssq, scalar=1.0 / W,
                                       in1=musq, op0=ALU.mult, op1=ALU.subtract)
        sd = self.p_stat.tile([128, n], dt.float32, tag="sd")
        nc.scalar.activation(sd, var, AF.Sqrt, bias=self.eps)
        rstd = self.p_stat.tile([128, n], dt.float32, tag="rstd")
        nc.vector.reciprocal(rstd, sd)
        nmr = self.p_stat.tile([128, n], dt.float32, tag="nmr")
        nc.vector.scalar_tensor_tensor(out=nmr, in0=mu, scalar=-1.0, in1=rstd,
                                       op0=ALU.mult, op1=ALU.mult)
        return rstd, nmr

    def ln_apply(self, out, z, rstd_col, nmr_col, gelu, gb_name=None):
        nc = self.nc
        general = gb_name is not None and not _affine_identity(self.arrays[gb_name])
        if not general:
            func = AF.Gelu if gelu else AF.Identity
            nc.scalar.activation(out, z, func, scale=rstd_col, bias=nmr_col)
            return
        g, b = self.arrays[gb_name]
        gname, bname = gb_name + "_g", gb_name + "_b"
        if gname not in self.arrays:
            self.arrays[gname] = _repl(g)
            self.arrays[bname] = _repl(b)
        W = out.shape[-1]
        gt = self.stream_weight(gname, f"s_lng_{W}")
        bt = self.stream_weight(bname, f"s_lnb_{W}")
        t = self.p_act1.tile([128, W], dt.float32, tag="lnt")
        nc.scalar.activation(t, z, AF.Identity, scale=rstd_col, bias=nmr_col)
        t2 = self.p_act1.tile([128, W], dt.float32, tag="lnt2")
        nc.vector.tensor_tensor(out=t2, in0=t, in1=gt, op=ALU.mult)
        t3 = self.p_act1.tile([128, W], dt.float32, tag="lnt3")
        nc.vector.tensor_tensor(out=t3, in0=t2, in1=bt, op=ALU.add)
        nc.scalar.activation(out, t3, AF.Gelu if gelu else AF.Copy)


# ----------------------------------------------------------------------------
# Attention helpers (shared by per-node rounds and global attention)
# ----------------------------------------------------------------------------

def _node_scores(em, featsf, qt, n, s_all, kwt, kname):
    """k projections for query-node n (weights kwt or resident kname) +
    scores into s_all[:, n]."""
    nc = em.nc
    ksb = em.p_big.tile([128, NN * H], dt.bfloat16, tag="kvsb")
    for half in range(2):
        psk = em.p_ps.tile([128, 1024], dt.float32, tag="mm")
        for j4 in range(4):
            j = half * 4 + j4
            em.mm(psk[:, j4 * H:(j4 + 1) * H], featsf[j], kname, wt=kwt)
        nc.vector.tensor_copy(out=ksb[:, half * 4 * H:(half + 1) * 4 * H],
                              in_=psk[:, :4 * H])
    qv = qt[:, n * H:(n + 1) * H].rearrange("p (a d) -> p a d", a=HEADS)
    qv = qv.unsqueeze(2).broadcast_to([128, HEADS, NN, DH])
    kvv = ksb.rearrange("p (j a d) -> p a j d", j=NN, a=HEADS)
    t = em.p_big.tile([128, HEADS, NN, DH], dt.bfloat16, tag="mscr")
    nc.vector.tensor_tensor(out=t, in0=qv, in1=kvv, op=ALU.mult)
    nc.vector.tensor_reduce(out=s_all[:, n], in_=t,
                            axis=mybir.AxisListType.X, op=ALU.add)


def _softmax(em, s_all, nq):
    nc = em.nc
    e = em.p_big.tile([128, nq, HEADS, NN], dt.bfloat16, tag="esm")
    nc.scalar.activation(e, s_all, AF.Exp, scale=SCALE)
    den = em.p_big.tile([128, nq * HEADS], dt.float32, tag="den")
    nc.vector.tensor_reduce(out=den, in_=e, axis=mybir.AxisListType.X,
                            op=ALU.add)
    rden = em.p_big.tile([128, nq * HEADS], dt.float32, tag="rden")
    nc.vector.reciprocal(rden, den)
    attn = em.p_big.tile([128, nq, HEADS, NN], dt.bfloat16, tag="attn")
    rv = rden.rearrange("p (n a) -> p n a", n=nq).unsqueeze(3).broadcast_to(
        [128, nq, HEADS, NN])
    nc.vector.tensor_tensor(out=attn, in0=e, in1=rv, op=ALU.mult)
    return attn


def _mix(em, featsf, attn_n, wv_name, bv_name, tag, wt=None):
    """V projections of all 8 kv-nodes with wv_name + attn-weighted mix.
    attn_n: [128, HEADS, NN]. Returns m_bf [128, H] bf16."""
    nc = em.nc
    vsb = em.p_big.tile([128, NN * H], dt.bfloat16, tag="kvsb")
    for half in range(2):
        psv = em.p_ps.tile([128, 1024], dt.float32, tag="mm")
        for j4 in range(4):
            j = half * 4 + j4
            em.mm(psv[:, j4 * H:(j4 + 1) * H], featsf[j], wv_name, wt=wt)
        nc.vector.tensor_copy(out=vsb[:, half * 4 * H:(half + 1) * 4 * H],
                              in_=psv[:, :4 * H])
    vv = vsb.rearrange("p (j a d) -> p a d j", j=NN, a=HEADS)
    av = attn_n.unsqueeze(2).broadcast_to([128, HEADS, DH, NN])
    t2 = em.p_big.tile([128, HEADS, DH, NN], dt.bfloat16, tag="mscr")
    nc.gpsimd.tensor_tensor(out=t2, in0=vv, in1=av, op=ALU.mult)
    m = em.p_act1.tile([128, H], dt.float32, tag="mixm")
    nc.vector.tensor_reduce(out=m.rearrange("p (a d) -> p a d", a=HEADS),
                            in_=t2, axis=mybir.AxisListType.X, op=ALU.add)
    if not _zero(em.arrays[bv_name]):
        nm = bv_name + "_repl"
        if nm not in em.arrays:
            em.arrays[nm] = _repl(em.arrays[bv_name].astype(F32)[0, :H])
        m2 = em.p_big.tile([128, H], dt.float32, tag="mixm2")
        nc.vector.tensor_tensor(out=m2, in0=m, in1=em.stream_weight(nm, "s_bvr"),
                                op=ALU.add)
        m = m2
    m_bf = em.p_big.tile([128, H], dt.bfloat16, tag="mixmb")
    nc.gpsimd.tensor_copy(out=m_bf, in_=m)
    return m_bf


# ----------------------------------------------------------------------------
# Stages
# ----------------------------------------------------------------------------

def _emit_round(em, feats, rnd):
    nc = em.nc
    featsf = [em.to_f(feats[n], f"ff{n}", pool=em.p_feat) for n in range(NN)]

    qt = em.p_act1.tile([128, NN * H], dt.bfloat16, tag="qtr")
    s_all = em.p_act1.tile([128, NN, HEADS, NN], dt.float32, tag="sall")
    for n in range(NN):
        wqk = em.stage_weight(f'wqk_{n}')
        psq = em.p_ps.tile([128, 1024], dt.float32, tag="mm")
        em.mm(psq[:, :H], featsf[n], f'wq_{n}', bias=f'bq_{n}',
              wt=wqk[:, :2 * H])
        nc.vector.tensor_copy(out=qt[:, n * H:(n + 1) * H], in_=psq[:, :H])
        _node_scores(em, featsf, qt, n, s_all, wqk[:, 2 * H:], f'wk_{n}')
    attn = _softmax(em, s_all, NN)

    gdot = em.p_stat.tile([128, NN], dt.float32, tag="gdot")
    s0 = em.p_stat.tile([128, NN], dt.float32, tag="ln0_s")
    q0 = em.p_stat.tile([128, NN], dt.float32, tag="ln0_q")
    sg = em.p_stat.tile([128, NN], dt.float32, tag="gate_s")
    qg = em.p_stat.tile([128, NN], dt.float32, tag="gate_q")
    gz_list, o_list = [], []
    for n in range(NN):
        wvo = em.stage_weight(f'wvo_{n}')
        m_bf = _mix(em, featsf, attn[:, n], f'wv_{n}', f'bv_{n}', "r",
                    wt=wvo[:, :2 * H])
        mf = em.to_f(m_bf, "mf")
        pso = em.p_ps.tile([128, 1024], dt.float32, tag="mm")
        em.mm(pso[:, :H], mf, f'wo_{n}', bias=f'bo_{n}', wt=wvo[:, 2 * H:])
        o_sb = em.p_feat.tile([128, H], dt.bfloat16, tag=f"o{n}")
        nc.vector.tensor_copy(out=o_sb, in_=pso[:, :H])
        o_list.append(o_sb)

        of = em.to_f(o_sb, "of")
        psg = em.p_ps.tile([128, 1024], dt.float32, tag="mm")
        zg = psg[:, :H + 1]
        _, _, neff = em.arrays[f'wg1_{n}']
        wt = em.stage_weight(f'wg1_{n}')
        first = True
        if not _zero(em.arrays[f'bg1_{n}']):
            nc.tensor.matmul(zg, lhsT=em.ones_row,
                             rhs=em.stage_weight(f'bg1_{n}'),
                             start=True, stop=False)
            first = False
        for c in range(4):
            src = featsf[n] if c < 2 else of
            cc = c % 2
            nc.tensor.matmul(zg, lhsT=src[:, cc * 128:(cc + 1) * 128],
                             rhs=wt[:, c * neff:(c + 1) * neff],
                             start=first, stop=(c == 3))
            first = False
        em.ln_stats(zg, H, sg[:, n:n + 1], qg[:, n:n + 1])
        gz = em.p_feat.tile([128, H], dt.bfloat16, tag=f"gz{n}")
        nc.vector.tensor_copy(out=gz, in_=zg[:, :H])
        gz_list.append(gz)

    rstd, nmr = em.ln_finalize(sg, qg, H, NN)
    for n in range(NN):
        gzb = em.p_feat.tile([128, H], dt.bfloat16, tag=f"gzb{n}")
        em.ln_apply(gzb, gz_list[n], rstd[:, n:n + 1], nmr[:, n:n + 1],
                    gelu=True, gb_name=f'gg1_{n}')
        dscr = em.p_scr.tile([128, H], dt.bfloat16, tag="dot")
        nc.vector.scalar_tensor_tensor(out=dscr, in0=gzb, scalar=1.0,
                                       in1=em.stage_weight(f'wg2_{n}'),
                                       op0=ALU.bypass, op1=ALU.mult,
                                       accum_out=gdot[:, n:n + 1])
    gate = em.p_stat.tile([128, NN], dt.float32, tag="gate")
    if _zero(em.arrays['bg2']):
        nc.scalar.activation(gate, gdot, AF.Sigmoid)
    else:
        gsum = em.p_stat.tile([128, NN], dt.float32, tag="gatesum")
        nc.vector.tensor_tensor(out=gsum, in0=gdot,
                                in1=em.stage_weight('bg2'), op=ALU.add)
        nc.scalar.activation(gate, gsum, AF.Sigmoid)

    xs = []
    for n in range(NN):
        x = em.p_feat.tile([128, H], dt.float32, tag=f"x{n}")
        nc.gpsimd.scalar_tensor_tensor(out=x, in0=o_list[n],
                                       scalar=gate[:, n:n + 1], in1=feats[n],
                                       op0=ALU.mult, op1=ALU.add,
                                       accum_out=s0[:, n:n + 1])
        nc.scalar.activation(em.sqscr(H), x, AF.Square,
                             accum_out=q0[:, n:n + 1])
        xs.append(x)
    rstd, nmr = em.ln_finalize(s0, q0, H, NN)
    new_feats = []
    for n in range(NN):
        fn = em.p_feat.tile([128, H], dt.bfloat16, tag=f"nf{rnd % 2}_{n}")
        nc.scalar.activation(fn, xs[n], AF.Identity, scale=rstd[:, n:n + 1],
                             bias=nmr[:, n:n + 1])
        new_feats.append(fn)
    return new_feats


def _emit_global(em, feats):
    nc = em.nc
    featsf = [em.to_f(feats[n], f"ff{n}", pool=em.p_feat) for n in range(NN)]
    acc = em.p_act1.tile([128, H], dt.float32, tag="qgacc")
    nc.vector.tensor_tensor(out=acc, in0=feats[0], in1=feats[1], op=ALU.add)
    for n in range(2, NN):
        nc.vector.tensor_tensor(out=acc, in0=acc, in1=feats[n], op=ALU.add)
    qg = em.p_act.tile([128, H], dt.bfloat16, tag="qg")
    nc.vector.tensor_scalar(out=qg, in0=acc, scalar1=1.0 / NN, scalar2=None,
                            op0=ALU.mult)
    qgf = em.to_f(qg, "qgf")
    psq = em.p_ps.tile([128, 1024], dt.float32, tag="mm")
    em.mm(psq[:, :H], qgf, 'wgq', bias='bgq')
    qt = em.p_act.tile([128, H], dt.bfloat16, tag="gqt")
    nc.vector.tensor_copy(out=qt, in_=psq[:, :H])

    s_all = em.p_big.tile([128, 1, HEADS, NN], dt.float32, tag="gsall")
    _node_scores(em, featsf, qt, 0, s_all, None, 'wgk')
    attn = _softmax(em, s_all, 1)
    m_bf = _mix(em, featsf, attn[:, 0], 'wgv', 'bgv', "g")

    mf = em.to_f(m_bf, "gmf")
    pso = em.p_ps.tile([128, 1024], dt.float32, tag="mm")
    em.mm(pso[:, :H + 1], mf, 'wgo', bias='bgo')
    state = em.p_act.tile([128, H], dt.float32, tag="state")
    nc.vector.tensor_copy(out=state, in_=pso[:, :H])
    st_sum = em.p_stat.tile([128, 1], dt.float32, tag="stsum")
    nc.vector.tensor_copy(out=st_sum, in_=pso[:, H:H + 1])
    return state, st_sum


def _emit_reasoning(em, state, st_sum, l):
    nc = em.nc
    ssq = em.p_stat.tile([128, 1], dt.float32, tag="rs_q1")
    nc.scalar.activation(em.sqscr(H), state, AF.Square, accum_out=ssq)
    rstd, nmr = em.ln_finalize(st_sum, ssq, H, 1)
    h1 = em.p_act.tile([128, H], dt.bfloat16, tag="rs_h1")
    em.ln_apply(h1, state, rstd[:, 0:1], nmr[:, 0:1], gelu=False,
                gb_name=f'rs_g1_{l}')
    rsw = em.stream_weight(f'rs_{l}', 's_rs')
    w_vo_w = em.arrays[f'rs_wvo_{l}'][1] * em.arrays[f'rs_wvo_{l}'][2]
    w_f1_w = em.arrays[f'rs_wf1_{l}'][1] * em.arrays[f'rs_wf1_{l}'][2]
    h1f = em.to_f(h1, "rs_h1f")
    psa = em.p_ps.tile([128, 1024], dt.float32, tag="mm")
    em.mm(psa[:, :H], h1f, f'rs_wvo_{l}', bias=f'rs_bvo_{l}',
          wt=rsw[:, :w_vo_w])
    s1 = em.p_act1.tile([128, H], dt.float32, tag="rs_s1")
    s1_sum = em.p_stat.tile([128, 1], dt.float32, tag="rs_s1s")
    nc.vector.scalar_tensor_tensor(out=s1, in0=psa[:, :H], scalar=1.0,
                                   in1=state, op0=ALU.bypass, op1=ALU.add,
                                   accum_out=s1_sum)
    ssq2 = em.p_stat.tile([128, 1], dt.float32, tag="rs_q2")
    nc.scalar.activation(em.sqscr(H), s1, AF.Square, accum_out=ssq2)
    rstd, nmr = em.ln_finalize(s1_sum, ssq2, H, 1)
    h2 = em.p_act.tile([128, H], dt.bfloat16, tag="rs_h2")
    em.ln_apply(h2, s1, rstd[:, 0:1], nmr[:, 0:1], gelu=False,
                gb_name=f'rs_g2_{l}')
    h2f = em.to_f(h2, "rs_h2f")
    psf = em.p_ps.tile([128, 1024], dt.float32, tag="mm")
    em.mm(psf, h2f, f'rs_wf1_{l}', bias=f'rs_bf1_{l}',
          wt=rsw[:, w_vo_w:w_vo_w + w_f1_w])
    ff = em.p_act1.tile([128, 4 * H], dt.bfloat16, tag="rs_ff")
    nc.scalar.activation(ff, psf, AF.Gelu)
    fff = em.to_f(ff, "rs_fff", pool=em.p_act1)
    psf2 = em.p_ps.tile([128, 1024], dt.float32, tag="mm")
    em.mm(psf2[:, :H], fff, f'rs_wf2_{l}', bias=f'rs_bf2_{l}',
          wt=rsw[:, w_vo_w + w_f1_w:])
    t = em.p_act1.tile([128, H], dt.float32, tag="rs_t")
    nc.vector.scalar_tensor_tensor(out=t, in0=psf2[:, :H], scalar=1.0, in1=s1,
                                   op0=ALU.bypass, op1=ALU.add)
    state2 = em.p_act.tile([128, H], dt.float32, tag="state")
    st_sum2 = em.p_stat.tile([128, 1], dt.float32, tag="stsum")
    nc.vector.scalar_tensor_tensor(out=state2, in0=t, scalar=1.0, in1=state,
                                   op0=ALU.bypass, op1=ALU.add,
                                   accum_out=st_sum2)
    return state2, st_sum2


def _emit_head(em, state, st_sum, y_d, r0):
    nc = em.nc
    sb = em.p_act.tile([128, H], dt.bfloat16, tag="hd_sb")
    nc.gpsimd.tensor_copy(out=sb, in_=state)
    sf = em.to_f(sb, "hd_sf")
    ps1 = em.p_ps.tile([128, 1024], dt.float32, tag="mm")
    z1 = ps1[:, :H + 1]
    em.mm(z1, sf, 'wo1', bias='bo1')
    s = em.p_stat.tile([128, 2], dt.float32, tag="hd_s")
    q = em.p_stat.tile([128, 2], dt.float32, tag="hd_q")
    em.ln_stats(z1, H, s[:, 0:1], q[:, 0:1])
    rstd, nmr = em.ln_finalize(s[:, 0:1], q[:, 0:1], H, 1)
    y1 = em.p_act.tile([128, H], dt.bfloat16, tag="hd_y1")
    em.ln_apply(y1, z1[:, :H], rstd[:, 0:1], nmr[:, 0:1], gelu=True,
                gb_name='go1')
    # y2 = y1 @ Wo2.T + bo2  (no LN)
    y1f = em.to_f(y1, "hd_y1f")
    ps2 = em.p_ps.tile([128, 1024], dt.float32, tag="mm")
    em.mm(ps2[:, :H // 2], y1f, 'wo2', bias='bo2')
    y2 = em.p_act.tile([128, H // 2], dt.bfloat16, tag="hd_y2")
    nc.vector.tensor_copy(out=y2, in_=ps2[:, :H // 2])
    # y3 = gelu(LN(y2 @ Wf1.T + bf1))
    y2f = em.to_f(y2, "hd_y2f")
    ps3 = em.p_ps.tile([128, 1024], dt.float32, tag="mm")
    z3 = ps3[:, :H // 2 + 1]
    em.mm(z3, y2f, 'wf1', bias='bf1')
    em.ln_stats(z3, H // 2, s[:, 1:2], q[:, 1:2])
    rstd, nmr = em.ln_finalize(s[:, 1:2], q[:, 1:2], H // 2, 1)
    y3 = em.p_act.tile([128, H // 2], dt.bfloat16, tag="hd_y3")
    em.ln_apply(y3, z3[:, :H // 2], rstd[:, 0:1], nmr[:, 0:1], gelu=True,
                gb_name='gf1')
    # y = y3 . wf2 + bf2
    ydot = em.p_stat.tile([128, 1], dt.float32, tag="ydot")
    dscr = em.p_scr.tile([128, H // 2], dt.bfloat16, tag="dot")
    nc.vector.scalar_tensor_tensor(out=dscr, in0=y3,
                                   scalar=1.0, in1=em.stage_weight('wf2'),
                                   op0=ALU.bypass, op1=ALU.mult,
                                   accum_out=ydot)
    y_sb = em.p_act.tile([128, 1], dt.float32, tag="hd_y")
    nc.scalar.activation(y_sb, ydot, AF.Copy, bias=float(em.arrays['bf2']))
    nc.sync.dma_start(out=y_d[r0:r0 + PT, :], in_=y_sb)


def _emit_tile(em, x_d, y_d, ti):
    nc = em.nc
    r0 = ti * PT

    xin = em.p_act1.tile([128, D], dt.float32, tag="xin")
    nc.sync.dma_start(out=xin, in_=x_d[r0:r0 + PT, :])
    xb = em.p_act1.tile([128, D], dt.bfloat16, tag="xb")
    nc.gpsimd.tensor_copy(out=xb, in_=xin)

    xf = em.to_f(xb, "xf", pool=em.p_act1)
    ps = em.p_ps.tile([128, 1024], dt.float32, tag="mm")
    z = ps[:, :2 * H + 1]
    em.mm(z, xf, 'we1', bias='be1')
    s1 = em.p_stat.tile([128, 2], dt.float32, tag="emb_s")
    q1 = em.p_stat.tile([128, 2], dt.float32, tag="emb_q")
    em.ln_stats(z, 2 * H, s1[:, 0:1], q1[:, 0:1])
    rstd, nmr = em.ln_finalize(s1[:, 0:1], q1[:, 0:1], 2 * H, 1)
    h1 = em.p_act1.tile([128, 2 * H], dt.bfloat16, tag="h1")
    em.ln_apply(h1, z[:, :2 * H], rstd[:, 0:1], nmr[:, 0:1], gelu=True,
                gb_name='ge1')

    h1f = em.to_f(h1, "h1f", pool=em.p_act1)
    ps2 = em.p_ps.tile([128, 1024], dt.float32, tag="mm")
    z2 = ps2[:, :H + 1]
    em.mm(z2, h1f, 'we2', bias='be2')
    em.ln_stats(z2, H, s1[:, 1:2], q1[:, 1:2])
    rstd, nmr = em.ln_finalize(s1[:, 1:2], q1[:, 1:2], H, 1)
    h = em.p_act.tile([128, H], dt.bfloat16, tag="h")
    em.ln_apply(h, z2[:, :H], rstd[:, 0:1], nmr[:, 0:1], gelu=True,
                gb_name='ge2')

    hf = em.to_f(h, "hf")
    feats = []
    NG = 2
    for grp in range(NN // NG):
        zs = []
        wns = []
        sgr = em.p_stat.tile([128, NG], dt.float32, tag="nd_s")
        qgr = em.p_stat.tile([128, NG], dt.float32, tag="nd_q")
        for i in range(NG):
            n = grp * NG + i
            psn = em.p_ps.tile([128, 1024], dt.float32, tag="mm")
            zn = psn[:, :2 * H + 1]
            wn = em.stream_weight(f'wn_{n}', 's_wn')
            wns.append(wn)
            w1w = em.arrays[f'wn1_{n}'][1] * em.arrays[f'wn1_{n}'][2]
            em.mm(zn, hf, f'wn1_{n}', bias=f'bn1_{n}', wt=wn[:, :w1w])
            em.ln_stats(zn, 2 * H, sgr[:, i:i + 1], qgr[:, i:i + 1])
            zs.append(zn)
        rstd, nmr = em.ln_finalize(sgr, qgr, 2 * H, NG)
        z1s = []
        for i in range(NG):
            n = grp * NG + i
            z1n = em.p_act1.tile([128, 2 * H], dt.bfloat16, tag=f"z1_{i}")
            em.ln_apply(z1n, zs[i][:, :2 * H], rstd[:, i:i + 1],
                        nmr[:, i:i + 1], gelu=True, gb_name=f'gn1_{n}')
            z1s.append(z1n)
        sg2 = em.p_stat.tile([128, NG], dt.float32, tag="nd_s2")
        qg2 = em.p_stat.tile([128, NG], dt.float32, tag="nd_q2")
        zps = []
        for i in range(NG):
            n = grp * NG + i
            z1f = em.to_f(z1s[i], f"z1f_{i}", pool=em.p_act1)
            psn = em.p_ps.tile([128, 1024], dt.float32, tag="mm")
            zn = psn[:, :H + 1]
            w1w = em.arrays[f'wn1_{n}'][1] * em.arrays[f'wn1_{n}'][2]
            em.mm(zn, z1f, f'wn2_{n}', bias=f'bn2_{n}', wt=wns[i][:, w1w:])
            em.ln_stats(zn, H, sg2[:, i:i + 1], qg2[:, i:i + 1])
            zps.append(zn)
        rstd, nmr = em.ln_finalize(sg2, qg2, H, NG)
        for i in range(NG):
            n = grp * NG + i
            fn = em.p_feat.tile([128, H], dt.bfloat16, tag=f"feat{n}")
            em.ln_apply(fn, zps[i][:, :H], rstd[:, i:i + 1], nmr[:, i:i + 1],
                        gelu=True, gb_name=f'gn2_{n}')
            feats.append(fn)

    for rnd in range(3):
        feats = _emit_round(em, feats, rnd)

    state, st_sum = _emit_global(em, feats)
    for l in range(L):
        state, st_sum = _emit_reasoning(em, state, st_sum, l)
    _emit_head(em, state, st_sum, y_d, r0)


# ----------------------------------------------------------------------------
# Program build + run
# ----------------------------------------------------------------------------

def build_program(arrays, b_core=BC):
    from contextlib import ExitStack
    nc = bacc.Bacc("TRN2", target_bir_lowering=False, debug=False)
    ntiles = b_core // PT
    with tile.TileContext(nc) as tc:
        with ExitStack() as ctx:
            em = Emitter(nc, tc, ctx, arrays)
            x_d = em.add_input("x", np.zeros((b_core, D), F32))
            y_h = nc.dram_tensor("y", [b_core, 1], dt.float32,
                                 kind="ExternalOutput")
            y_d = y_h.ap()
            for t in range(ntiles):
                _emit_tile(em, x_d, y_d, t)
    nc.compile()
    # input name -> host array (weights); 'x' filled per core at run time
    wmap = {}
    for name, ap in em.dram.items():
        if name == 'x':
            continue
        arr = arrays[name[2:]]
        if isinstance(arr, tuple):
            arr = arr[0]
        wmap[name] = np.ascontiguousarray(arr)
    return nc, wmap


_CACHE = {}


def kernel(x, params):
    x = np.asarray(x, F32)
    assert x.shape == (B, D), x.shape
    arrays = _prep(params)
    key = "prog"
    if key not in _CACHE:
        _CACHE[key] = build_program(arrays, BC)
    nc, wmap = _CACHE[key]
    in_maps = []
    for c in range(NCORES):
        m = dict(wmap)
        m['x'] = np.ascontiguousarray(x[c * BC:(c + 1) * BC])
        in_maps.append(m)
    res = bass_utils.run_bass_kernel_spmd(nc, in_maps,
                                          core_ids=list(range(NCORES)))
    out = np.concatenate([res.results[c]['y'] for c in range(NCORES)], axis=0)
    return out.astype(F32)


# revision 33
# speedup vs baseline: 1.2080x; 1.2080x over previous
"""Trainium2 Bass kernel for nn_AdvancedWebOfThoughts (gnn_message_passing).

Strategy:
  - Pure data parallel over 8 NeuronCores (1024 batch rows each), 8 tiles of
    128 rows per core.
  - Batch-major activations [128 batch partitions, features]; every weight
    matmul runs with the activation (feature-major, PE-transposed) as the
    stationary operand and the bf16 weight matrix as the moving operand, so
    matmul outputs land batch-major in PSUM.
  - LayerNorm: sum(z) rides the matmul as an extra weight column (w-bar);
    sum(z^2) comes from a ScalarE Square pass with accum_out; normalize
    (+ optional gelu) is one fused ScalarE activation with per-partition
    scale/bias.
  - Message passing attention: explicit per-(node, kv-node) K/V projections
    on the TensorEngine; scores / softmax / mix are batched DVE
    multiply + grouped-reduce ops in batch-major layout.

kernel() is self-contained: shapes hardcoded, all weight preprocessing is
host-side numpy.
"""

import sys

sys.path.insert(0, "/opt/trn_rl_repo")

import numpy as np
import ml_dtypes

import concourse.bass as bass  # noqa: F401
import concourse.bacc as bacc
import concourse.tile as tile
from concourse import mybir
from concourse.masks import make_identity
from concourse import bass_utils

BF16 = ml_dtypes.bfloat16
F32 = np.float32

D, H, NN, L, HEADS, T_, B = 768, 256, 8, 4, 4, 32, 8192
DH = H // HEADS
TASK = 2
EPS = 1e-5
NCORES = 8
BC = B // NCORES
PT = 128
SCALE = 1.0 / np.sqrt(DH)

AF = mybir.ActivationFunctionType
ALU = mybir.AluOpType
dt = mybir.dt


# ----------------------------------------------------------------------------
# Host-side weight prep
# ----------------------------------------------------------------------------

def _bf(a):
    return np.ascontiguousarray(np.asarray(a, F32)).astype(BF16)


def _pack_rhs(w, wbar=False):
    """w: [out_features, in_features] fp32 (torch Linear layout) ->
    (packed [128, nchunk*neff] bf16, nchunk, neff) in moving-operand layout."""
    wt = np.asarray(w, F32).T  # [K, N]
    K, N = wt.shape
    assert K % 128 == 0, (K, N)
    wt_bf = wt.astype(BF16)
    if wbar:
        col = wt_bf.astype(F32).sum(axis=1, keepdims=True)
        wt_bf = np.concatenate([wt_bf, col.astype(BF16)], axis=1)
        N += 1
    chunks = [wt_bf[c * 128:(c + 1) * 128] for c in range(K // 128)]
    return (np.ascontiguousarray(np.concatenate(chunks, axis=1)), K // 128, N)


def _bias_row(b, wbar=False):
    b = np.asarray(b, F32).reshape(1, -1)
    if wbar:
        b = np.concatenate([b, b.sum(axis=1, keepdims=True)], axis=1)
    return np.ascontiguousarray(b.astype(BF16))


def _repl(v):
    v = np.asarray(v, F32).reshape(1, -1)
    return np.ascontiguousarray(np.tile(v.astype(BF16), (128, 1)))


def _prep(params):
    p = {k: np.asarray(v, F32) for k, v in params.items()}
    a = {}
    a['we1'] = _pack_rhs(p['W_e1'], wbar=True)
    a['be1'] = _bias_row(p['b_e1'], wbar=True)
    a['ge1'] = (p['g_e1'], p['be_e1'])
    a['we2'] = _pack_rhs(p['W_e2'], wbar=True)
    a['be2'] = _bias_row(p['b_e2'], wbar=True)
    a['ge2'] = (p['g_e2'], p['be_e2'])

    te = p['task_emb'][TASK]
    for n in range(NN):
        w1 = p['Wn1'][n]
        b1 = p['bn1'][n] + w1[:, H:] @ te
        a[f'wn1_{n}'] = _pack_rhs(w1[:, :H], wbar=True)
        a[f'bn1_{n}'] = _bias_row(b1, wbar=True)
        a[f'gn1_{n}'] = (p['gn1'][n], p['bln1'][n])
        a[f'wn2_{n}'] = _pack_rhs(p['Wn2'][n], wbar=True)
        a[f'wn_{n}'] = (np.ascontiguousarray(np.concatenate(
            [a[f'wn1_{n}'][0], a[f'wn2_{n}'][0]], axis=1)), 0, 0)
        a[f'bn2_{n}'] = _bias_row(p['bn2'][n], wbar=True)
        a[f'gn2_{n}'] = (p['gn2'][n], p['bln2'][n])

        wq_p = _pack_rhs(p['Wea_in'][n][:H])
        wk_p = _pack_rhs(p['Wea_in'][n][H:2 * H])
        wv_p = _pack_rhs(p['Wea_in'][n][2 * H:])
        wo_p = _pack_rhs(p['Wea_out'][n])
        a[f'wq_{n}'] = wq_p
        a[f'wk_{n}'] = wk_p
        a[f'wv_{n}'] = wv_p
        a[f'wqk_{n}'] = (np.ascontiguousarray(
            np.concatenate([wq_p[0], wk_p[0]], axis=1)), 2, 256)
        a[f'wvo_{n}'] = (np.ascontiguousarray(
            np.concatenate([wv_p[0], wo_p[0]], axis=1)), 2, 256)
        a[f'bq_{n}'] = _bias_row(p['bea_in'][n][:H])
        a[f'bv_{n}'] = _bias_row(p['bea_in'][n][2 * H:])
        a[f'wo_{n}'] = _pack_rhs(p['Wea_out'][n])
        a[f'bo_{n}'] = _bias_row(p['bea_out'][n])

        a[f'wg1_{n}'] = _pack_rhs(p['Wg1'][n], wbar=True)
        a[f'bg1_{n}'] = _bias_row(p['bg1'][n], wbar=True)
        a[f'gg1_{n}'] = (p['gg1'][n], p['bgl1'][n])
        a[f'wg2_{n}'] = _repl(p['Wg2'][n][0])
    a['bg2'] = np.ascontiguousarray(
        np.tile(p['bg2'].reshape(1, NN).astype(F32), (128, 1)))

    a['wgq'] = _pack_rhs(p['Wga_in'][:H])
    a['bgq'] = _bias_row(p['bga_in'][:H])
    a['wgk'] = _pack_rhs(p['Wga_in'][H:2 * H])
    a['wgv'] = _pack_rhs(p['Wga_in'][2 * H:])
    a['bgv'] = _bias_row(p['bga_in'][2 * H:])
    a['wgo'] = _pack_rhs(p['Wga_out'], wbar=True)
    a['bgo'] = _bias_row(p['bga_out'], wbar=True)

    for l in range(L):
        wv_l = p['rs_Win'][l][2 * H:]
        bv_l = p['rs_bin'][l][2 * H:]
        w_vo = p['rs_Wout'][l] @ wv_l
        b_vo = p['rs_Wout'][l] @ bv_l + p['rs_bout'][l]
        a[f'rs_wvo_{l}'] = _pack_rhs(w_vo)
        a[f'rs_bvo_{l}'] = _bias_row(b_vo)
        a[f'rs_wf1a_{l}'] = _pack_rhs(p['rs_Wf1'][l][:2 * H])
        a[f'rs_wf1b_{l}'] = _pack_rhs(p['rs_Wf1'][l][2 * H:])
        a[f'rs_bf1a_{l}'] = _bias_row(p['rs_bf1'][l][:2 * H])
        a[f'rs_bf1b_{l}'] = _bias_row(p['rs_bf1'][l][2 * H:])
        a[f'rs_wf2_{l}'] = _pack_rhs(p['rs_Wf2'][l])
        a[f'rs_bf2_{l}'] = _bias_row(p['rs_bf2'][l])
        a[f'rs_{l}'] = (np.ascontiguousarray(np.concatenate(
            [a[f'rs_wvo_{l}'][0], a[f'rs_wf1a_{l}'][0], a[f'rs_wf1b_{l}'][0],
             a[f'rs_wf2_{l}'][0]], axis=1)), 0, 0)
        a[f'rs_g1_{l}'] = (p['rs_g1'][l], p['rs_b1'][l])
        a[f'rs_g2_{l}'] = (p['rs_g2'][l], p['rs_b2'][l])

    a['wo1'] = _pack_rhs(p['Wo1'], wbar=True)
    a['bo1'] = _bias_row(p['bo1'], wbar=True)
    a['go1'] = (p['go1'], p['bol1'])
    a['wo2'] = _pack_rhs(p['Wo2'])          # no LN after Wo2
    a['bo2'] = _bias_row(p['bo2'])
    a['wf1'] = _pack_rhs(p['Wf1'], wbar=True)
    a['bf1'] = _bias_row(p['bf1'], wbar=True)
    a['gf1'] = (p['gf1'], p['bfl1'])
    a['wf2'] = _repl(p['Wf2'][0])
    a['bf2'] = float(np.asarray(p['bf2']).reshape(-1)[0])
    return a


def _affine_identity(gb):
    g, b = gb
    return np.allclose(g, 1.0) and np.allclose(b, 0.0)


def _zero(row):
    return not np.any(np.asarray(row, F32))


# ----------------------------------------------------------------------------
# Emitter
# ----------------------------------------------------------------------------

class Emitter:
    def __init__(self, nc, tc, ctx, arrays):
        self.nc = nc
        self.tc = tc
        self.arrays = arrays
        self.dram = {}
        self.wsb = {}

        self.p_w = ctx.enter_context(tc.tile_pool(name="wts", bufs=1))
        self.p_ws = ctx.enter_context(tc.tile_pool(name="wstream", bufs=2))
        self.p_ws1 = ctx.enter_context(tc.tile_pool(name="wstream1", bufs=1))
        self.p_const = ctx.enter_context(tc.tile_pool(name="const", bufs=1))
        self.p_act = ctx.enter_context(tc.tile_pool(name="act", bufs=2))
        self.p_act1 = ctx.enter_context(tc.tile_pool(name="act1", bufs=1))
        self.p_feat = ctx.enter_context(tc.tile_pool(name="feat", bufs=1))
        self.p_big = ctx.enter_context(tc.tile_pool(name="big", bufs=2))
        self.p_stat = ctx.enter_context(tc.tile_pool(name="stat", bufs=3))
        self.p_psW = ctx.enter_context(
            tc.tile_pool(name="psW", bufs=2, space="PSUM"))
        self.p_psS = ctx.enter_context(
            tc.tile_pool(name="psS", bufs=3, space="PSUM"))
        self.p_ps_tr = ctx.enter_context(
            tc.tile_pool(name="pstr", bufs=1, space="PSUM"))

        self.ident = self.p_const.tile([128, 128], dt.bfloat16, tag="ident")
        make_identity(nc, self.ident)
        self.ones_row = self.p_const.tile([1, 128], dt.bfloat16, tag="ones")
        nc.vector.memset(self.ones_row, 1.0)
        self.eps = self.p_const.tile([128, 1], dt.float32, tag="eps")
        nc.vector.memset(self.eps, EPS)
        self.p_scr = ctx.enter_context(tc.tile_pool(name="scr", bufs=2))

    def add_input(self, name, arr):
        h = self.nc.dram_tensor(name, list(arr.shape), dt.from_np(arr.dtype),
                                kind="ExternalInput")
        ap = h.ap()
        self.dram[name] = ap
        return ap

    def stage_weight(self, name):
        if name in self.wsb:
            return self.wsb[name]
        arr = self.arrays[name]
        if isinstance(arr, tuple):
            arr = arr[0]
        d = self.add_input("w_" + name, arr)
        t = self.p_w.tile(list(arr.shape), dt.from_np(arr.dtype), tag="w_" + name)
        self.nc.sync.dma_start(out=t, in_=d)
        self.wsb[name] = t
        return t

    def stream_weight(self, name, tag, bufs1=False):
        """Per-tile staged weight (re-DMA'd each use; slot shared via tag)."""
        arr = self.arrays[name]
        if isinstance(arr, tuple):
            arr = arr[0]
        if ("w_" + name) not in self.dram:
            self.add_input("w_" + name, arr)
        d = self.dram["w_" + name]
        pool = self.p_ws1 if bufs1 else self.p_ws
        t = pool.tile(list(arr.shape), dt.from_np(arr.dtype), tag=tag)
        self.nc.sync.dma_start(out=t, in_=d)
        return t

    def mm(self, psum, actf, wname, bias=None, stream_tag=None, wt=None):
        nc = self.nc
        _, nchunk, neff = self.arrays[wname]
        if wt is None:
            if stream_tag is not None:
                wt = self.stream_weight(wname, stream_tag)
            else:
                wt = self.stage_weight(wname)
        segs = []
        off = 0
        while off < neff:
            seg = min(512, neff - off)
            segs.append((off, seg))
            off += seg
        have_bias = bias is not None and not _zero(self.arrays[bias])
        bt = self.stream_weight(bias, "s_brow") if have_bias else None
        for (off, seg) in segs:
            first = True
            if have_bias:
                nc.tensor.matmul(psum[:, off:off + seg], lhsT=self.ones_row,
                                 rhs=bt[:, off:off + seg], start=True,
                                 stop=False)
                first = False
            for c in range(nchunk):
                nc.tensor.matmul(psum[:, off:off + seg],
                                 lhsT=actf[:, c * 128:(c + 1) * 128],
                                 rhs=wt[:, c * neff + off:c * neff + off + seg],
                                 start=first, stop=(c == nchunk - 1))
                first = False

    def to_f(self, src, tag, pool=None):
        nc = self.nc
        W = src.shape[-1]
        f = (pool or self.p_act).tile([128, W], dt.bfloat16, tag=tag)
        nch = W // 128
        c = 0
        while c < nch:
            g = min(4, nch - c)
            tp = self.p_ps_tr.tile([128, 512], dt.bfloat16, tag="trp")
            for i in range(g):
                nc.tensor.transpose(tp[:, i * 128:(i + 1) * 128],
                                    src[:, (c + i) * 128:(c + i + 1) * 128],
                                    self.ident)
            nc.vector.tensor_copy(out=f[:, c * 128:(c + g) * 128],
                                  in_=tp[:, :g * 128])
            c += g
        return f

    def sqscr(self, W):
        sq_scratch = self.p_scr.tile([128, W], dt.bfloat16, tag="sq")
        return sq_scratch

    def ln_stats(self, z_ps, W, sums, ssq, col=True):
        nc = self.nc
        if col:
            nc.vector.tensor_copy(out=sums, in_=z_ps[:, W:W + 1])
        else:
            cp1 = self.sqscr(W)
            nc.scalar.activation(cp1, z_ps[:, :W], AF.Copy, accum_out=sums)
        sq1 = self.sqscr(W)
        nc.scalar.activation(sq1, z_ps[:, :W], AF.Square, accum_out=ssq)

    def ln_finalize(self, sums, ssq, W, n):
        nc = self.nc
        mu = self.p_stat.tile([128, n], dt.float32, tag="mu")
        nc.vector.tensor_scalar(out=mu, in0=sums, scalar1=1.0 / W, scalar2=None,
                                op0=ALU.mult)
        musq = self.p_stat.tile([128, n], dt.float32, tag="musq")
        nc.vector.tensor_tensor(out=musq, in0=mu, in1=mu, op=ALU.mult)
        var = self.p_stat.tile([128, n], dt.float32, tag="var")
        nc.vector.scalar_tensor_tensor(out=var, in0=ssq, scalar=1.0 / W,
                                       in1=musq, op0=ALU.mult, op1=ALU.subtract)
        sd = self.p_stat.tile([128, n], dt.float32, tag="sd")
        nc.scalar.activation(sd, var, AF.Sqrt, bias=self.eps)
        rstd = self.p_stat.tile([128, n], dt.float32, tag="rstd")
        nc.vector.reciprocal(rstd, sd)
        nmr = self.p_stat.tile([128, n], dt.float32, tag="nmr")
        nc.vector.scalar_tensor_tensor(out=nmr, in0=mu, scalar=-1.0, in1=rstd,
                                       op0=ALU.mult, op1=ALU.mult)
        return rstd, nmr

    def ln_apply(self, out, z, rstd_col, nmr_col, gelu, gb_name=None):
        nc = self.nc
        general = gb_name is not None and not _affine_identity(self.arrays[gb_name])
        if not general:
            if gelu:
                nc.scalar.activation(out, z, AF.Gelu, scale=rstd_col,
                                     bias=nmr_col)
            elif z.space == bass.MemorySpace.PSUM:
                nc.scalar.activation(out, z, AF.Identity, scale=rstd_col,
                                     bias=nmr_col)
            else:
                nc.vector.tensor_scalar(out=out, in0=z, scalar1=rstd_col,
                                        scalar2=nmr_col, op0=ALU.mult,
                                        op1=ALU.add)
            return
        g, b = self.arrays[gb_name]
        gname, bname = gb_name + "_g", gb_name + "_b"
        if gname not in self.arrays:
            self.arrays[gname] = _repl(g)
            self.arrays[bname] = _repl(b)
        W = out.shape[-1]
        gt = self.stream_weight(gname, f"s_lng_{W}")
        bt = self.stream_weight(bname, f"s_lnb_{W}")
        t = self.p_act1.tile([128, W], dt.float32, tag="lnt")
        nc.scalar.activation(t, z, AF.Identity, scale=rstd_col, bias=nmr_col)
        t2 = self.p_act1.tile([128, W], dt.float32, tag="lnt2")
        nc.vector.tensor_tensor(out=t2, in0=t, in1=gt, op=ALU.mult)
        t3 = self.p_act1.tile([128, W], dt.float32, tag="lnt3")
        nc.vector.tensor_tensor(out=t3, in0=t2, in1=bt, op=ALU.add)
        nc.scalar.activation(out, t3, AF.Gelu if gelu else AF.Copy)


# ----------------------------------------------------------------------------
# Attention helpers (shared by per-node rounds and global attention)
# ----------------------------------------------------------------------------

def _node_scores(em, featsf, qt, n, s_all, kwt, kname):
    """k projections for query-node n (weights kwt or resident kname) +
    scores into s_all[:, n]."""
    nc = em.nc
    ksb = em.p_big.tile([128, NN * H], dt.bfloat16, tag="kvsb")
    for half in range(2):
        psk = em.p_psW.tile([128, 1024], dt.float32, tag="mmW")
        for j4 in range(4):
            j = half * 4 + j4
            em.mm(psk[:, j4 * H:(j4 + 1) * H], featsf[j], kname, wt=kwt)
        nc.scalar.copy(ksb[:, half * 4 * H:(half + 1) * 4 * H],
                       psk[:, :4 * H])
    qv = qt[:, n * H:(n + 1) * H].rearrange("p (a d) -> p a d", a=HEADS)
    qv = qv.unsqueeze(2).broadcast_to([128, HEADS, NN, DH])
    kvv = ksb.rearrange("p (j a d) -> p a j d", j=NN, a=HEADS)
    t = em.p_big.tile([128, HEADS, NN, DH], dt.bfloat16, tag="kvsb")
    nc.gpsimd.tensor_tensor(out=t, in0=qv, in1=kvv, op=ALU.mult)
    nc.vector.tensor_reduce(out=s_all[:, n], in_=t,
                            axis=mybir.AxisListType.X, op=ALU.add)


def _softmax(em, s_all, nq):
    nc = em.nc
    attn = em.p_big.tile([128, nq, HEADS, NN], dt.bfloat16, tag="attn")
    nc.scalar.activation(attn, s_all, AF.Exp, scale=SCALE)
    den = em.p_big.tile([128, nq * HEADS], dt.float32, tag="den")
    nc.vector.tensor_reduce(out=den, in_=attn, axis=mybir.AxisListType.X,
                            op=ALU.add)
    rden = den
    nc.vector.reciprocal(rden, den)
    rv = rden.rearrange("p (n a) -> p n a", n=nq).unsqueeze(3).broadcast_to(
        [128, nq, HEADS, NN])
    nc.vector.tensor_tensor(out=attn, in0=attn, in1=rv, op=ALU.mult)
    return attn


def _mix(em, featsf, attn_n, wv_name, bv_name, tag, wt=None):
    """V projections of all 8 kv-nodes with wv_name + attn-weighted mix.
    attn_n: [128, HEADS, NN]. Returns m_bf [128, H] bf16."""
    nc = em.nc
    vsb = em.p_big.tile([128, NN * H], dt.bfloat16, tag="kvsb")
    for half in range(2):
        psv = em.p_psW.tile([128, 1024], dt.float32, tag="mmW")
        for j4 in range(4):
            j = half * 4 + j4
            em.mm(psv[:, j4 * H:(j4 + 1) * H], featsf[j], wv_name, wt=wt)
        nc.scalar.copy(vsb[:, half * 4 * H:(half + 1) * 4 * H],
                       psv[:, :4 * H])
    vv = vsb.rearrange("p (j a d) -> p a d j", j=NN, a=HEADS)
    av = attn_n.unsqueeze(2).broadcast_to([128, HEADS, DH, NN])
    t2 = em.p_big.tile([128, HEADS, DH, NN], dt.bfloat16, tag="kvsb")
    nc.gpsimd.tensor_tensor(out=t2, in0=vv, in1=av, op=ALU.mult)
    m = em.p_act1.tile([128, H], dt.float32, tag="mixm")
    nc.vector.tensor_reduce(out=m.rearrange("p (a d) -> p a d", a=HEADS),
                            in_=t2, axis=mybir.AxisListType.X, op=ALU.add)
    if not _zero(em.arrays[bv_name]):
        nm = bv_name + "_repl"
        if nm not in em.arrays:
            em.arrays[nm] = _repl(em.arrays[bv_name].astype(F32)[0, :H])
        m2 = em.p_big.tile([128, H], dt.float32, tag="mixm2")
        nc.vector.tensor_tensor(out=m2, in0=m, in1=em.stream_weight(nm, "s_bvr"),
                                op=ALU.add)
        m = m2
    m_bf = em.p_big.tile([128, H], dt.bfloat16, tag="mixmb")
    nc.gpsimd.tensor_copy(out=m_bf, in_=m)
    return m_bf


# ----------------------------------------------------------------------------
# Stages
# ----------------------------------------------------------------------------

def _emit_round(em, feats, rnd, s):
    nc = em.nc
    featsf = [em.to_f(feats[n], f"ff{n}{s}", pool=em.p_feat) for n in range(NN)]

    qt = em.p_big.tile([128, NN * H], dt.bfloat16, tag="qtr")
    s_all = em.p_big.tile([128, NN, HEADS, NN], dt.float32, tag="sall")
    for n in range(NN):
        wqk = em.stage_weight(f'wqk_{n}')
        psq = em.p_psS.tile([128, 512], dt.float32, tag="mmS")
        em.mm(psq[:, :H], featsf[n], f'wq_{n}', bias=f'bq_{n}',
              wt=wqk[:, :2 * H])
        nc.vector.tensor_copy(out=qt[:, n * H:(n + 1) * H], in_=psq[:, :H])
        _node_scores(em, featsf, qt, n, s_all, wqk[:, 2 * H:], f'wk_{n}')
    attn = _softmax(em, s_all, NN)

    gdot = em.p_stat.tile([128, NN], dt.float32, tag="gdot")
    s0 = em.p_stat.tile([128, NN], dt.float32, tag="ln0_s")
    q0 = em.p_stat.tile([128, NN], dt.float32, tag="ln0_q")
    sg = em.p_stat.tile([128, NN], dt.float32, tag="gate_s")
    qg = em.p_stat.tile([128, NN], dt.float32, tag="gate_q")
    gz_list, o_list = [], []
    for n in range(NN):
        wvo = em.stage_weight(f'wvo_{n}')
        m_bf = _mix(em, featsf, attn[:, n], f'wv_{n}', f'bv_{n}', "r",
                    wt=wvo[:, :2 * H])
        mf = em.to_f(m_bf, "mf")
        pso = em.p_psS.tile([128, 512], dt.float32, tag="mmS")
        em.mm(pso[:, :H], mf, f'wo_{n}', bias=f'bo_{n}', wt=wvo[:, 2 * H:])
        o_sb = em.p_feat.tile([128, H], dt.bfloat16, tag=f"o{n}{s}")
        nc.vector.tensor_copy(out=o_sb, in_=pso[:, :H])
        o_list.append(o_sb)

        of = em.to_f(o_sb, "of")
        psg = em.p_psS.tile([128, 512], dt.float32, tag="mmS")
        zg = psg[:, :H + 1]
        _, _, neff = em.arrays[f'wg1_{n}']
        wt = em.stage_weight(f'wg1_{n}')
        first = True
        if not _zero(em.arrays[f'bg1_{n}']):
            nc.tensor.matmul(zg, lhsT=em.ones_row,
                             rhs=em.stage_weight(f'bg1_{n}'),
                             start=True, stop=False)
            first = False
        for c in range(4):
            src = featsf[n] if c < 2 else of
            cc = c % 2
            nc.tensor.matmul(zg, lhsT=src[:, cc * 128:(cc + 1) * 128],
                             rhs=wt[:, c * neff:(c + 1) * neff],
                             start=first, stop=(c == 3))
            first = False
        em.ln_stats(zg, H, sg[:, n:n + 1], qg[:, n:n + 1])
        gz = em.p_feat.tile([128, H], dt.bfloat16, tag=f"gz{n}{s}")
        nc.vector.tensor_copy(out=gz, in_=zg[:, :H])
        gz_list.append(gz)

    rstd, nmr = em.ln_finalize(sg, qg, H, NN)
    for n in range(NN):
        gzb = gz_list[n]
        em.ln_apply(gzb, gz_list[n], rstd[:, n:n + 1], nmr[:, n:n + 1],
                    gelu=True, gb_name=f'gg1_{n}')
        dscr = em.p_scr.tile([128, H], dt.bfloat16, tag="dot")
        nc.vector.scalar_tensor_tensor(out=dscr, in0=gzb, scalar=1.0,
                                       in1=em.stage_weight(f'wg2_{n}'),
                                       op0=ALU.bypass, op1=ALU.mult,
                                       accum_out=gdot[:, n:n + 1])
    gate = em.p_stat.tile([128, NN], dt.float32, tag="gate")
    if _zero(em.arrays['bg2']):
        nc.scalar.activation(gate, gdot, AF.Sigmoid)
    else:
        gsum = em.p_stat.tile([128, NN], dt.float32, tag="gatesum")
        nc.vector.tensor_tensor(out=gsum, in0=gdot,
                                in1=em.stage_weight('bg2'), op=ALU.add)
        nc.scalar.activation(gate, gsum, AF.Sigmoid)

    xs = []
    for n in range(NN):
        x = em.p_feat.tile([128, H], dt.bfloat16, tag=f"x{n}{s}")
        nc.vector.scalar_tensor_tensor(out=x, in0=o_list[n],
                                       scalar=gate[:, n:n + 1], in1=feats[n],
                                       op0=ALU.mult, op1=ALU.add,
                                       accum_out=s0[:, n:n + 1])
        sq2 = em.sqscr(H)
        nc.scalar.activation(sq2, x, AF.Square, accum_out=q0[:, n:n + 1])
        xs.append(x)
    rstd, nmr = em.ln_finalize(s0, q0, H, NN)
    new_feats = []
    for n in range(NN):
        fn = em.p_feat.tile([128, H], dt.bfloat16, tag=f"nf{rnd % 2}_{n}{s}")
        nc.vector.tensor_scalar(out=fn, in0=xs[n], scalar1=rstd[:, n:n + 1],
                                scalar2=nmr[:, n:n + 1], op0=ALU.mult,
                                op1=ALU.add)
        new_feats.append(fn)
    return new_feats


def _emit_global(em, feats, s):
    nc = em.nc
    featsf = [em.to_f(feats[n], f"ff{n}{s}", pool=em.p_feat) for n in range(NN)]
    acc = em.p_act1.tile([128, H], dt.float32, tag=f"rs_t{s}")
    nc.vector.tensor_tensor(out=acc, in0=feats[0], in1=feats[1], op=ALU.add)
    for n in range(2, NN):
        nc.vector.tensor_tensor(out=acc, in0=acc, in1=feats[n], op=ALU.add)
    qg = em.p_act.tile([128, H], dt.bfloat16, tag="qg")
    nc.vector.tensor_scalar(out=qg, in0=acc, scalar1=1.0 / NN, scalar2=None,
                            op0=ALU.mult)
    qgf = em.to_f(qg, "qgf")
    psq = em.p_psS.tile([128, 512], dt.float32, tag="mmS")
    em.mm(psq[:, :H], qgf, 'wgq', bias='bgq')
    qt = em.p_act.tile([128, H], dt.bfloat16, tag="gqt")
    nc.vector.tensor_copy(out=qt, in_=psq[:, :H])

    s_all = em.p_big.tile([128, 1, HEADS, NN], dt.float32, tag="gsall")
    _node_scores(em, featsf, qt, 0, s_all, None, 'wgk')
    attn = _softmax(em, s_all, 1)
    m_bf = _mix(em, featsf, attn[:, 0], 'wgv', 'bgv', "g")

    mf = em.to_f(m_bf, "gmf")
    pso = em.p_psS.tile([128, 512], dt.float32, tag="mmS")
    em.mm(pso[:, :H + 1], mf, 'wgo', bias='bgo')
    state = em.p_act.tile([128, H], dt.float32, tag=f"state{s}")
    nc.vector.tensor_copy(out=state, in_=pso[:, :H])
    st_sum = em.p_stat.tile([128, 1], dt.float32, tag=f"stsum{s}")
    nc.vector.tensor_copy(out=st_sum, in_=pso[:, H:H + 1])
    return state, st_sum


def _emit_reasoning(em, state, st_sum, l, s, rsw=None):
    nc = em.nc
    ssq = em.p_stat.tile([128, 1], dt.float32, tag="rs_q1")
    sq3 = em.sqscr(H)
    nc.scalar.activation(sq3, state, AF.Square, accum_out=ssq)
    rstd, nmr = em.ln_finalize(st_sum, ssq, H, 1)
    h1 = em.p_act.tile([128, H], dt.bfloat16, tag="rs_h1")
    em.ln_apply(h1, state, rstd[:, 0:1], nmr[:, 0:1], gelu=False,
                gb_name=f'rs_g1_{l}')
    if rsw is None:
        rsw = em.stream_weight(f'rs_{l}', 's_rs', bufs1=True)
    w_vo_w = em.arrays[f'rs_wvo_{l}'][1] * em.arrays[f'rs_wvo_{l}'][2]
    w_f1_w = 2 * em.arrays[f'rs_wf1a_{l}'][1] * em.arrays[f'rs_wf1a_{l}'][2]
    h1f = em.to_f(h1, "rs_h1f")
    psa = em.p_psS.tile([128, 512], dt.float32, tag="mmS")
    em.mm(psa[:, :H], h1f, f'rs_wvo_{l}', bias=f'rs_bvo_{l}',
          wt=rsw[:, :w_vo_w])
    s1 = em.p_act1.tile([128, H], dt.float32, tag=f"rs_s1{s}")
    s1_sum = em.p_stat.tile([128, 1], dt.float32, tag="rs_s1s")
    nc.vector.scalar_tensor_tensor(out=s1, in0=psa[:, :H], scalar=1.0,
                                   in1=state, op0=ALU.bypass, op1=ALU.add,
                                   accum_out=s1_sum)
    ssq2 = em.p_stat.tile([128, 1], dt.float32, tag="rs_q2")
    sq4 = em.sqscr(H)
    nc.scalar.activation(sq4, s1, AF.Square, accum_out=ssq2)
    rstd, nmr = em.ln_finalize(s1_sum, ssq2, H, 1)
    h2 = em.p_act.tile([128, H], dt.bfloat16, tag="rs_h2")
    em.ln_apply(h2, s1, rstd[:, 0:1], nmr[:, 0:1], gelu=False,
                gb_name=f'rs_g2_{l}')
    h2f = em.to_f(h2, "rs_h2f")
    ff = em.p_act1.tile([128, 4 * H], dt.bfloat16, tag="rs_ff")
    wfa = em.arrays[f'rs_wf1a_{l}'][1] * em.arrays[f'rs_wf1a_{l}'][2]
    psf = em.p_psW.tile([128, 1024], dt.float32, tag="mmW")
    for hh in range(2):
        em.mm(psf[:, hh * 512:(hh + 1) * 512], h2f,
              f'rs_wf1a_{l}' if hh == 0 else f'rs_wf1b_{l}',
              bias=f'rs_bf1a_{l}' if hh == 0 else f'rs_bf1b_{l}',
              wt=rsw[:, w_vo_w + hh * wfa:w_vo_w + (hh + 1) * wfa])
    nc.scalar.activation(ff, psf, AF.Gelu)
    fff = em.to_f(ff, "rs_fff", pool=em.p_act1)
    psf2 = em.p_psS.tile([128, 512], dt.float32, tag="mmS")
    em.mm(psf2[:, :H], fff, f'rs_wf2_{l}', bias=f'rs_bf2_{l}',
          wt=rsw[:, w_vo_w + w_f1_w:])
    t = em.p_act1.tile([128, H], dt.float32, tag=f"rs_t{s}")
    nc.vector.scalar_tensor_tensor(out=t, in0=psf2[:, :H], scalar=1.0, in1=s1,
                                   op0=ALU.bypass, op1=ALU.add)
    state2 = em.p_act.tile([128, H], dt.float32, tag=f"state{s}")
    st_sum2 = em.p_stat.tile([128, 1], dt.float32, tag=f"stsum{s}")
    nc.vector.scalar_tensor_tensor(out=state2, in0=t, scalar=1.0, in1=state,
                                   op0=ALU.bypass, op1=ALU.add,
                                   accum_out=st_sum2)
    return state2, st_sum2


def _emit_head(em, state, st_sum, y_d, r0, s):
    nc = em.nc
    sb = em.p_act.tile([128, H], dt.bfloat16, tag="hd_sb")
    nc.gpsimd.tensor_copy(out=sb, in_=state)
    sf = em.to_f(sb, "hd_sf")
    ps1 = em.p_psS.tile([128, 512], dt.float32, tag="mmS")
    z1 = ps1[:, :H + 1]
    em.mm(z1, sf, 'wo1', bias='bo1')
    s = em.p_stat.tile([128, 2], dt.float32, tag="hd_s")
    q = em.p_stat.tile([128, 2], dt.float32, tag="hd_q")
    em.ln_stats(z1, H, s[:, 0:1], q[:, 0:1])
    rstd, nmr = em.ln_finalize(s[:, 0:1], q[:, 0:1], H, 1)
    y1 = em.p_act.tile([128, H], dt.bfloat16, tag="hd_y1")
    em.ln_apply(y1, z1[:, :H], rstd[:, 0:1], nmr[:, 0:1], gelu=True,
                gb_name='go1')
    # y2 = y1 @ Wo2.T + bo2  (no LN)
    y1f = em.to_f(y1, "hd_y1f")
    ps2 = em.p_psS.tile([128, 512], dt.float32, tag="mmS")
    em.mm(ps2[:, :H // 2], y1f, 'wo2', bias='bo2')
    y2 = em.p_act.tile([128, H // 2], dt.bfloat16, tag="hd_y2")
    nc.vector.tensor_copy(out=y2, in_=ps2[:, :H // 2])
    # y3 = gelu(LN(y2 @ Wf1.T + bf1))
    y2f = em.to_f(y2, "hd_y2f")
    ps3 = em.p_psS.tile([128, 512], dt.float32, tag="mmS")
    z3 = ps3[:, :H // 2 + 1]
    em.mm(z3, y2f, 'wf1', bias='bf1')
    em.ln_stats(z3, H // 2, s[:, 1:2], q[:, 1:2])
    rstd, nmr = em.ln_finalize(s[:, 1:2], q[:, 1:2], H // 2, 1)
    y3 = em.p_act.tile([128, H // 2], dt.bfloat16, tag="hd_y3")
    em.ln_apply(y3, z3[:, :H // 2], rstd[:, 0:1], nmr[:, 0:1], gelu=True,
                gb_name='gf1')
    # y = y3 . wf2 + bf2
    ydot = em.p_stat.tile([128, 1], dt.float32, tag="ydot")
    dscr = em.p_scr.tile([128, H // 2], dt.bfloat16, tag="dot")
    nc.vector.scalar_tensor_tensor(out=dscr, in0=y3,
                                   scalar=1.0, in1=em.stage_weight('wf2'),
                                   op0=ALU.bypass, op1=ALU.mult,
                                   accum_out=ydot)
    y_sb = em.p_act.tile([128, 1], dt.float32, tag="hd_y")
    nc.scalar.activation(y_sb, ydot, AF.Copy, bias=float(em.arrays['bf2']))
    nc.sync.dma_start(out=y_d[r0:r0 + PT, :], in_=y_sb)


def _stage_load_embed(em, st):
    nc = em.nc
    s = st['s']
    xb = em.p_act1.tile([128, D], dt.bfloat16, tag="xb")
    nc.sync.dma_start(out=xb, in_=st['x_d'][st['r0']:st['r0'] + PT, :])
    xf = em.to_f(xb, "xf")
    ps = em.p_psW.tile([128, 1024], dt.float32, tag="mmW")
    z = ps[:, :2 * H + 1]
    em.mm(z, xf, 'we1', bias='be1')
    s1 = em.p_stat.tile([128, 2], dt.float32, tag="emb_s")
    q1 = em.p_stat.tile([128, 2], dt.float32, tag="emb_q")
    em.ln_stats(z, 2 * H, s1[:, 0:1], q1[:, 0:1])
    rstd, nmr = em.ln_finalize(s1[:, 0:1], q1[:, 0:1], 2 * H, 1)
    h1 = em.p_act.tile([128, 2 * H], dt.bfloat16, tag="h1")
    em.ln_apply(h1, z[:, :2 * H], rstd[:, 0:1], nmr[:, 0:1], gelu=True,
                gb_name='ge1')

    h1f = em.to_f(h1, "h1f")
    ps2 = em.p_psS.tile([128, 512], dt.float32, tag="mmS")
    z2 = ps2[:, :H + 1]
    em.mm(z2, h1f, 'we2', bias='be2')
    em.ln_stats(z2, H, s1[:, 1:2], q1[:, 1:2])
    rstd, nmr = em.ln_finalize(s1[:, 1:2], q1[:, 1:2], H, 1)
    h = em.p_act.tile([128, H], dt.bfloat16, tag="h")
    em.ln_apply(h, z2[:, :H], rstd[:, 0:1], nmr[:, 0:1], gelu=True,
                gb_name='ge2')
    st['h'] = h


def _stage_nodes(em, st):
    nc = em.nc
    s2 = st['s']
    hf = em.to_f(st['h'], "hf")
    feats = []
    NG = 2
    for grp in range(NN // NG):
        zs = []
        wns = []
        sgr = em.p_stat.tile([128, NG], dt.float32, tag="nd_s")
        qgr = em.p_stat.tile([128, NG], dt.float32, tag="nd_q")
        for i in range(NG):
            n = grp * NG + i
            psn = em.p_psW.tile([128, 1024], dt.float32, tag="mmW")
            zn = psn[:, :2 * H + 1]
            wn = em.stream_weight(f'wn_{n}', 's_wn')
            wns.append(wn)
            w1w = em.arrays[f'wn1_{n}'][1] * em.arrays[f'wn1_{n}'][2]
            em.mm(zn, hf, f'wn1_{n}', bias=f'bn1_{n}', wt=wn[:, :w1w])
            em.ln_stats(zn, 2 * H, sgr[:, i:i + 1], qgr[:, i:i + 1])
            zs.append(zn)
        rstd, nmr = em.ln_finalize(sgr, qgr, 2 * H, NG)
        z1s = []
        for i in range(NG):
            n = grp * NG + i
            z1n = em.p_act.tile([128, 2 * H], dt.bfloat16, tag=f"z1_{i}")
            em.ln_apply(z1n, zs[i][:, :2 * H], rstd[:, i:i + 1],
                        nmr[:, i:i + 1], gelu=True, gb_name=f'gn1_{n}')
            z1s.append(z1n)
        sg2 = em.p_stat.tile([128, NG], dt.float32, tag="nd_s2")
        qg2 = em.p_stat.tile([128, NG], dt.float32, tag="nd_q2")
        zps = []
        for i in range(NG):
            n = grp * NG + i
            z1f = em.to_f(z1s[i], f"z1f_{i}")
            psn = em.p_psS.tile([128, 512], dt.float32, tag="mmS")
            zn = psn[:, :H + 1]
            w1w = em.arrays[f'wn1_{n}'][1] * em.arrays[f'wn1_{n}'][2]
            em.mm(zn, z1f, f'wn2_{n}', bias=f'bn2_{n}', wt=wns[i][:, w1w:])
            em.ln_stats(zn, H, sg2[:, i:i + 1], qg2[:, i:i + 1])
            zps.append(zn)
        rstd, nmr = em.ln_finalize(sg2, qg2, H, NG)
        for i in range(NG):
            n = grp * NG + i
            fn = em.p_feat.tile([128, H], dt.bfloat16, tag=f"nf1_{n}{s2}")
            em.ln_apply(fn, zps[i][:, :H], rstd[:, i:i + 1], nmr[:, i:i + 1],
                        gelu=True, gb_name=f'gn2_{n}')
            feats.append(fn)
    st['feats'] = feats


def _stage_round(em, st, rnd):
    st['feats'] = _emit_round(em, st['feats'], rnd, st['s'])


def _stage_global(em, st):
    state, st_sum = _emit_global(em, st['feats'], st['s'])
    st['state'] = state
    st['st_sum'] = st_sum


def _stage_reasoning(em, st, l, rsw=None):
    state, st_sum = _emit_reasoning(em, st['state'], st['st_sum'], l, st['s'],
                                    rsw=rsw)
    st['state'] = state
    st['st_sum'] = st_sum


def _stage_head(em, st):
    _emit_head(em, st['state'], st['st_sum'], st['y_d'], st['r0'], st['s'])


# ----------------------------------------------------------------------------
# Program build + run
# ----------------------------------------------------------------------------

def build_program(arrays, b_core=BC):
    from contextlib import ExitStack
    nc = bacc.Bacc("TRN2", target_bir_lowering=False, debug=False)
    ntiles = b_core // PT
    with tile.TileContext(nc) as tc:
        with ExitStack() as ctx:
            em = Emitter(nc, tc, ctx, arrays)
            x_d = em.add_input("x", np.zeros((b_core, D), BF16))
            y_h = nc.dram_tensor("y", [b_core, 1], dt.float32,
                                 kind="ExternalOutput")
            y_d = y_h.ap()
            GS = 2
            for t0 in range(0, ntiles, GS):
                pair = [dict(s=si, x_d=x_d, y_d=y_d, r0=(t0 + si) * PT)
                        for si in range(min(GS, ntiles - t0))]
                for st in pair:
                    _stage_load_embed(em, st)
                for st in pair:
                    _stage_nodes(em, st)
                for rnd in range(3):
                    for st in pair:
                        _stage_round(em, st, rnd)
                for st in pair:
                    _stage_global(em, st)
                for l in range(L):
                    rsw = em.stream_weight(f'rs_{l}', 's_rs', bufs1=True)
                    for st in pair:
                        _stage_reasoning(em, st, l, rsw)
                for st in pair:
                    _stage_head(em, st)
    nc.compile()
    # input name -> host array (weights); 'x' filled per core at run time
    wmap = {}
    for name, ap in em.dram.items():
        if name == 'x':
            continue
        arr = arrays[name[2:]]
        if isinstance(arr, tuple):
            arr = arr[0]
        wmap[name] = np.ascontiguousarray(arr)
    return nc, wmap


_CACHE = {}


def kernel(x, params):
    x = np.asarray(x, F32)
    assert x.shape == (B, D), x.shape
    arrays = _prep(params)
    key = "prog"
    if key not in _CACHE:
        # Note: program structure (zero-bias / identity-affine gates) is
        # specialized to the first call's params; weight VALUES are re-read
        # from `arrays` on every call below.
        _CACHE[key] = build_program(arrays, BC)
    nc, wmap = _CACHE[key]
    fresh = {}
    for name in wmap:
        arr = arrays.get(name[2:])
        if arr is None:
            fresh[name] = wmap[name]  # derived array created at build time
            continue
        if isinstance(arr, tuple):
            arr = arr[0]
        fresh[name] = np.ascontiguousarray(arr)
    in_maps = []
    for c in range(NCORES):
        m = dict(fresh)
        m['x'] = np.ascontiguousarray(x[c * BC:(c + 1) * BC]).astype(BF16)
        in_maps.append(m)
    res = bass_utils.run_bass_kernel_spmd(nc, in_maps,
                                          core_ids=list(range(NCORES)))
    out = np.concatenate([res.results[c]['y'] for c in range(NCORES)], axis=0)
    return out.astype(F32)


# revision 34
# speedup vs baseline: 77.1794x; 63.8928x over previous
"""Trainium2 Bass kernel for nn_AdvancedWebOfThoughts (gnn_message_passing).

Strategy:
  - Pure data parallel over 8 NeuronCores (1024 batch rows each), 8 tiles of
    128 rows per core.
  - Batch-major activations [128 batch partitions, features]; every weight
    matmul runs with the activation (feature-major, PE-transposed) as the
    stationary operand and the bf16 weight matrix as the moving operand, so
    matmul outputs land batch-major in PSUM.
  - LayerNorm: sum(z) rides the matmul as an extra weight column (w-bar);
    sum(z^2) comes from a ScalarE Square pass with accum_out; normalize
    (+ optional gelu) is one fused ScalarE activation with per-partition
    scale/bias.
  - Message passing attention: explicit per-(node, kv-node) K/V projections
    on the TensorEngine; scores / softmax / mix are batched DVE
    multiply + grouped-reduce ops in batch-major layout.

kernel() is self-contained: shapes hardcoded, all weight preprocessing is
host-side numpy.
"""

import sys

sys.path.insert(0, "/opt/trn_rl_repo")

import numpy as np
import ml_dtypes

import concourse.bass as bass  # noqa: F401
import concourse.bacc as bacc
import concourse.tile as tile
from concourse import mybir
from concourse.masks import make_identity
from concourse import bass_utils

BF16 = ml_dtypes.bfloat16
F32 = np.float32

D, H, NN, L, HEADS, T_, B = 768, 256, 8, 4, 4, 32, 8192
DH = H // HEADS
TASK = 2
EPS = 1e-5
NCORES = 8
BC = B // NCORES
PT = 128
SCALE = 1.0 / np.sqrt(DH)

AF = mybir.ActivationFunctionType
ALU = mybir.AluOpType
dt = mybir.dt


# ----------------------------------------------------------------------------
# Host-side weight prep
# ----------------------------------------------------------------------------

def _bf(a):
    return np.ascontiguousarray(np.asarray(a, F32)).astype(BF16)


def _pack_rhs(w, wbar=False):
    """w: [out_features, in_features] fp32 (torch Linear layout) ->
    (packed [128, nchunk*neff] bf16, nchunk, neff) in moving-operand layout."""
    wt = np.asarray(w, F32).T  # [K, N]
    K, N = wt.shape
    assert K % 128 == 0, (K, N)
    wt_bf = wt.astype(BF16)
    if wbar:
        col = wt_bf.astype(F32).sum(axis=1, keepdims=True)
        wt_bf = np.concatenate([wt_bf, col.astype(BF16)], axis=1)
        N += 1
    chunks = [wt_bf[c * 128:(c + 1) * 128] for c in range(K // 128)]
    return (np.ascontiguousarray(np.concatenate(chunks, axis=1)), K // 128, N)


def _bias_row(b, wbar=False):
    b = np.asarray(b, F32).reshape(1, -1)
    if wbar:
        b = np.concatenate([b, b.sum(axis=1, keepdims=True)], axis=1)
    return np.ascontiguousarray(b.astype(BF16))


def _repl(v):
    v = np.asarray(v, F32).reshape(1, -1)
    return np.ascontiguousarray(np.tile(v.astype(BF16), (128, 1)))


def _prep(params):
    p = {k: np.asarray(v, F32) for k, v in params.items()}
    a = {}
    a['we1'] = _pack_rhs(p['W_e1'], wbar=True)
    a['be1'] = _bias_row(p['b_e1'], wbar=True)
    a['ge1'] = (p['g_e1'], p['be_e1'])
    a['we2'] = _pack_rhs(p['W_e2'], wbar=True)
    a['be2'] = _bias_row(p['b_e2'], wbar=True)
    a['ge2'] = (p['g_e2'], p['be_e2'])

    te = p['task_emb'][TASK]
    for n in range(NN):
        w1 = p['Wn1'][n]
        b1 = p['bn1'][n] + w1[:, H:] @ te
        a[f'wn1_{n}'] = _pack_rhs(w1[:, :H], wbar=True)
        a[f'bn1_{n}'] = _bias_row(b1, wbar=True)
        a[f'gn1_{n}'] = (p['gn1'][n], p['bln1'][n])
        a[f'wn2_{n}'] = _pack_rhs(p['Wn2'][n], wbar=True)
        a[f'wn_{n}'] = (np.ascontiguousarray(np.concatenate(
            [a[f'wn1_{n}'][0], a[f'wn2_{n}'][0]], axis=1)), 0, 0)
        a[f'bn2_{n}'] = _bias_row(p['bn2'][n], wbar=True)
        a[f'gn2_{n}'] = (p['gn2'][n], p['bln2'][n])

        wq_p = _pack_rhs(p['Wea_in'][n][:H])
        wk_p = _pack_rhs(p['Wea_in'][n][H:2 * H])
        wv_p = _pack_rhs(p['Wea_in'][n][2 * H:])
        wo_p = _pack_rhs(p['Wea_out'][n])
        a[f'wq_{n}'] = wq_p
        a[f'wk_{n}'] = wk_p
        a[f'wv_{n}'] = wv_p
        a[f'wqk_{n}'] = (np.ascontiguousarray(
            np.concatenate([wq_p[0], wk_p[0]], axis=1)), 2, 256)
        a[f'wvo_{n}'] = (np.ascontiguousarray(
            np.concatenate([wv_p[0], wo_p[0]], axis=1)), 2, 256)
        a[f'bq_{n}'] = _bias_row(p['bea_in'][n][:H])
        a[f'bv_{n}'] = _bias_row(p['bea_in'][n][2 * H:])
        a[f'wo_{n}'] = _pack_rhs(p['Wea_out'][n])
        a[f'bo_{n}'] = _bias_row(p['bea_out'][n])

        a[f'wg1_{n}'] = _pack_rhs(p['Wg1'][n], wbar=True)
        a[f'bg1_{n}'] = _bias_row(p['bg1'][n], wbar=True)
        a[f'gg1_{n}'] = (p['gg1'][n], p['bgl1'][n])
        a[f'wg2_{n}'] = _repl(p['Wg2'][n][0])
    a['bg2'] = np.ascontiguousarray(
        np.tile(p['bg2'].reshape(1, NN).astype(F32), (128, 1)))

    a['wgq'] = _pack_rhs(p['Wga_in'][:H])
    a['bgq'] = _bias_row(p['bga_in'][:H])
    a['wgk'] = _pack_rhs(p['Wga_in'][H:2 * H])
    a['wgv'] = _pack_rhs(p['Wga_in'][2 * H:])
    a['bgv'] = _bias_row(p['bga_in'][2 * H:])
    a['wgo'] = _pack_rhs(p['Wga_out'], wbar=True)
    a['bgo'] = _bias_row(p['bga_out'], wbar=True)

    for l in range(L):
        wv_l = p['rs_Win'][l][2 * H:]
        bv_l = p['rs_bin'][l][2 * H:]
        w_vo = p['rs_Wout'][l] @ wv_l
        b_vo = p['rs_Wout'][l] @ bv_l + p['rs_bout'][l]
        a[f'rs_wvo_{l}'] = _pack_rhs(w_vo)
        a[f'rs_bvo_{l}'] = _bias_row(b_vo)
        a[f'rs_wf1a_{l}'] = _pack_rhs(p['rs_Wf1'][l][:2 * H])
        a[f'rs_wf1b_{l}'] = _pack_rhs(p['rs_Wf1'][l][2 * H:])
        a[f'rs_bf1a_{l}'] = _bias_row(p['rs_bf1'][l][:2 * H])
        a[f'rs_bf1b_{l}'] = _bias_row(p['rs_bf1'][l][2 * H:])
        a[f'rs_wf2_{l}'] = _pack_rhs(p['rs_Wf2'][l])
        a[f'rs_bf2_{l}'] = _bias_row(p['rs_bf2'][l])
        a[f'rs_{l}'] = (np.ascontiguousarray(np.concatenate(
            [a[f'rs_wvo_{l}'][0], a[f'rs_wf1a_{l}'][0], a[f'rs_wf1b_{l}'][0],
             a[f'rs_wf2_{l}'][0]], axis=1)), 0, 0)
        a[f'rs_g1_{l}'] = (p['rs_g1'][l], p['rs_b1'][l])
        a[f'rs_g2_{l}'] = (p['rs_g2'][l], p['rs_b2'][l])

    a['wo1'] = _pack_rhs(p['Wo1'], wbar=True)
    a['bo1'] = _bias_row(p['bo1'], wbar=True)
    a['go1'] = (p['go1'], p['bol1'])
    a['wo2'] = _pack_rhs(p['Wo2'])          # no LN after Wo2
    a['bo2'] = _bias_row(p['bo2'])
    a['wf1'] = _pack_rhs(p['Wf1'], wbar=True)
    a['bf1'] = _bias_row(p['bf1'], wbar=True)
    a['gf1'] = (p['gf1'], p['bfl1'])
    a['wf2'] = _repl(p['Wf2'][0])
    a['bf2'] = float(np.asarray(p['bf2']).reshape(-1)[0])
    return a


def _affine_identity(gb):
    g, b = gb
    return np.allclose(g, 1.0) and np.allclose(b, 0.0)


def _zero(row):
    return not np.any(np.asarray(row, F32))


# ----------------------------------------------------------------------------
# Emitter
# ----------------------------------------------------------------------------

class Emitter:
    def __init__(self, nc, tc, ctx, arrays):
        self.nc = nc
        self.tc = tc
        self.arrays = arrays
        self.dram = {}
        self.wsb = {}

        self.p_w = ctx.enter_context(tc.tile_pool(name="wts", bufs=1))
        self.p_ws = ctx.enter_context(tc.tile_pool(name="wstream", bufs=2))
        self.p_ws1 = ctx.enter_context(tc.tile_pool(name="wstream1", bufs=1))
        self.p_const = ctx.enter_context(tc.tile_pool(name="const", bufs=1))
        self.p_act = ctx.enter_context(tc.tile_pool(name="act", bufs=2))
        self.p_act1 = ctx.enter_context(tc.tile_pool(name="act1", bufs=1))
        self.p_feat = ctx.enter_context(tc.tile_pool(name="feat", bufs=1))
        self.p_big = ctx.enter_context(tc.tile_pool(name="big", bufs=2))
        self.p_stat = ctx.enter_context(tc.tile_pool(name="stat", bufs=3))
        self.p_psW = ctx.enter_context(
            tc.tile_pool(name="psW", bufs=2, space="PSUM"))
        self.p_psS = ctx.enter_context(
            tc.tile_pool(name="psS", bufs=3, space="PSUM"))
        self.p_ps_tr = ctx.enter_context(
            tc.tile_pool(name="pstr", bufs=1, space="PSUM"))

        self.ident = self.p_const.tile([128, 128], dt.bfloat16, tag="ident")
        make_identity(nc, self.ident)
        self.ones_row = self.p_const.tile([1, 128], dt.bfloat16, tag="ones")
        nc.vector.memset(self.ones_row, 1.0)
        self.eps = self.p_const.tile([128, 1], dt.float32, tag="eps")
        nc.vector.memset(self.eps, EPS)
        self.p_scr = ctx.enter_context(tc.tile_pool(name="scr", bufs=2))

    def add_input(self, name, arr):
        h = self.nc.dram_tensor(name, list(arr.shape), dt.from_np(arr.dtype),
                                kind="ExternalInput")
        ap = h.ap()
        self.dram[name] = ap
        return ap

    def stage_weight(self, name):
        if name in self.wsb:
            return self.wsb[name]
        arr = self.arrays[name]
        if isinstance(arr, tuple):
            arr = arr[0]
        d = self.add_input("w_" + name, arr)
        t = self.p_w.tile(list(arr.shape), dt.from_np(arr.dtype), tag="w_" + name)
        self.nc.sync.dma_start(out=t, in_=d)
        self.wsb[name] = t
        return t

    def stream_weight(self, name, tag, bufs1=False):
        """Per-tile staged weight (re-DMA'd each use; slot shared via tag)."""
        arr = self.arrays[name]
        if isinstance(arr, tuple):
            arr = arr[0]
        if ("w_" + name) not in self.dram:
            self.add_input("w_" + name, arr)
        d = self.dram["w_" + name]
        pool = self.p_ws1 if bufs1 else self.p_ws
        t = pool.tile(list(arr.shape), dt.from_np(arr.dtype), tag=tag)
        self.nc.sync.dma_start(out=t, in_=d)
        return t

    def mm(self, psum, actf, wname, bias=None, stream_tag=None, wt=None):
        nc = self.nc
        _, nchunk, neff = self.arrays[wname]
        if wt is None:
            if stream_tag is not None:
                wt = self.stream_weight(wname, stream_tag)
            else:
                wt = self.stage_weight(wname)
        segs = []
        off = 0
        while off < neff:
            seg = min(512, neff - off)
            segs.append((off, seg))
            off += seg
        have_bias = bias is not None and not _zero(self.arrays[bias])
        bt = self.stream_weight(bias, "s_brow") if have_bias else None
        for (off, seg) in segs:
            first = True
            if have_bias:
                nc.tensor.matmul(psum[:, off:off + seg], lhsT=self.ones_row,
                                 rhs=bt[:, off:off + seg], start=True,
                                 stop=False)
                first = False
            for c in range(nchunk):
                nc.tensor.matmul(psum[:, off:off + seg],
                                 lhsT=actf[:, c * 128:(c + 1) * 128],
                                 rhs=wt[:, c * neff + off:c * neff + off + seg],
                                 start=first, stop=(c == nchunk - 1))
                first = False

    def to_f(self, src, tag, pool=None):
        nc = self.nc
        W = src.shape[-1]
        f = (pool or self.p_act).tile([128, W], dt.bfloat16, tag=tag)
        nch = W // 128
        c = 0
        while c < nch:
            g = min(4, nch - c)
            tp = self.p_ps_tr.tile([128, 512], dt.bfloat16, tag="trp")
            for i in range(g):
                nc.tensor.transpose(tp[:, i * 128:(i + 1) * 128],
                                    src[:, (c + i) * 128:(c + i + 1) * 128],
                                    self.ident)
            nc.vector.tensor_copy(out=f[:, c * 128:(c + g) * 128],
                                  in_=tp[:, :g * 128])
            c += g
        return f

    def sqscr(self, W):
        sq_scratch = self.p_scr.tile([128, W], dt.bfloat16, tag="sq")
        return sq_scratch

    def ln_stats(self, z_ps, W, sums, ssq, col=True):
        nc = self.nc
        if col:
            nc.vector.tensor_copy(out=sums, in_=z_ps[:, W:W + 1])
        else:
            cp1 = self.sqscr(W)
            nc.scalar.activation(cp1, z_ps[:, :W], AF.Copy, accum_out=sums)
        sq1 = self.sqscr(W)
        nc.scalar.activation(sq1, z_ps[:, :W], AF.Square, accum_out=ssq)

    def ln_finalize(self, sums, ssq, W, n):
        nc = self.nc
        mu = self.p_stat.tile([128, n], dt.float32, tag="mu")
        nc.vector.tensor_scalar(out=mu, in0=sums, scalar1=1.0 / W, scalar2=None,
                                op0=ALU.mult)
        musq = self.p_stat.tile([128, n], dt.float32, tag="musq")
        nc.vector.tensor_tensor(out=musq, in0=mu, in1=mu, op=ALU.mult)
        var = self.p_stat.tile([128, n], dt.float32, tag="var")
        nc.vector.scalar_tensor_tensor(out=var, in0=ssq, scalar=1.0 / W,
                                       in1=musq, op0=ALU.mult, op1=ALU.subtract)
        sd = self.p_stat.tile([128, n], dt.float32, tag="sd")
        nc.scalar.activation(sd, var, AF.Sqrt, bias=self.eps)
        rstd = self.p_stat.tile([128, n], dt.float32, tag="rstd")
        nc.vector.reciprocal(rstd, sd)
        nmr = self.p_stat.tile([128, n], dt.float32, tag="nmr")
        nc.vector.scalar_tensor_tensor(out=nmr, in0=mu, scalar=-1.0, in1=rstd,
                                       op0=ALU.mult, op1=ALU.mult)
        return rstd, nmr

    def ln_apply(self, out, z, rstd_col, nmr_col, gelu, gb_name=None):
        nc = self.nc
        general = gb_name is not None and not _affine_identity(self.arrays[gb_name])
        if not general:
            if gelu:
                nc.scalar.activation(out, z, AF.Gelu, scale=rstd_col,
                                     bias=nmr_col)
            elif z.space == bass.MemorySpace.PSUM:
                nc.scalar.activation(out, z, AF.Identity, scale=rstd_col,
                                     bias=nmr_col)
            else:
                nc.vector.tensor_scalar(out=out, in0=z, scalar1=rstd_col,
                                        scalar2=nmr_col, op0=ALU.mult,
                                        op1=ALU.add)
            return
        g, b = self.arrays[gb_name]
        gname, bname = gb_name + "_g", gb_name + "_b"
        if gname not in self.arrays:
            self.arrays[gname] = _repl(g)
            self.arrays[bname] = _repl(b)
        W = out.shape[-1]
        gt = self.stream_weight(gname, f"s_lng_{W}")
        bt = self.stream_weight(bname, f"s_lnb_{W}")
        t = self.p_act1.tile([128, W], dt.float32, tag="lnt")
        nc.scalar.activation(t, z, AF.Identity, scale=rstd_col, bias=nmr_col)
        t2 = self.p_act1.tile([128, W], dt.float32, tag="lnt2")
        nc.vector.tensor_tensor(out=t2, in0=t, in1=gt, op=ALU.mult)
        t3 = self.p_act1.tile([128, W], dt.float32, tag="lnt3")
        nc.vector.tensor_tensor(out=t3, in0=t2, in1=bt, op=ALU.add)
        nc.scalar.activation(out, t3, AF.Gelu if gelu else AF.Copy)


# ----------------------------------------------------------------------------
# Attention helpers (shared by per-node rounds and global attention)
# ----------------------------------------------------------------------------

def _node_scores(em, featsf, qt, n, s_all, kwt, kname):
    """k projections for query-node n (weights kwt or resident kname) +
    scores into s_all[:, n]."""
    nc = em.nc
    ksb = em.p_big.tile([128, NN * H], dt.bfloat16, tag="kvsb")
    for half in range(2):
        psk = em.p_psW.tile([128, 1024], dt.float32, tag="mmW")
        for j4 in range(4):
            j = half * 4 + j4
            em.mm(psk[:, j4 * H:(j4 + 1) * H], featsf[j], kname, wt=kwt)
        nc.scalar.copy(ksb[:, half * 4 * H:(half + 1) * 4 * H],
                       psk[:, :4 * H])
    qv = qt[:, n * H:(n + 1) * H].rearrange("p (a d) -> p a d", a=HEADS)
    qv = qv.unsqueeze(2).broadcast_to([128, HEADS, NN, DH])
    kvv = ksb.rearrange("p (j a d) -> p a j d", j=NN, a=HEADS)
    t = em.p_big.tile([128, HEADS, NN, DH], dt.bfloat16, tag="kvsb")
    nc.gpsimd.tensor_tensor(out=t, in0=qv, in1=kvv, op=ALU.mult)
    nc.vector.tensor_reduce(out=s_all[:, n], in_=t,
                            axis=mybir.AxisListType.X, op=ALU.add)


def _softmax(em, s_all, nq):
    nc = em.nc
    attn = em.p_big.tile([128, nq, HEADS, NN], dt.bfloat16, tag="attn")
    nc.scalar.activation(attn, s_all, AF.Exp, scale=SCALE)
    den = em.p_big.tile([128, nq * HEADS], dt.float32, tag="den")
    nc.vector.tensor_reduce(out=den, in_=attn, axis=mybir.AxisListType.X,
                            op=ALU.add)
    rden = den
    nc.vector.reciprocal(rden, den)
    rv = rden.rearrange("p (n a) -> p n a", n=nq).unsqueeze(3).broadcast_to(
        [128, nq, HEADS, NN])
    nc.vector.tensor_tensor(out=attn, in0=attn, in1=rv, op=ALU.mult)
    return attn


def _mix(em, featsf, attn_n, wv_name, bv_name, tag, wt=None):
    """V projections of all 8 kv-nodes with wv_name + attn-weighted mix.
    attn_n: [128, HEADS, NN]. Returns m_bf [128, H] bf16."""
    nc = em.nc
    vsb = em.p_big.tile([128, NN * H], dt.bfloat16, tag="kvsb")
    for half in range(2):
        psv = em.p_psW.tile([128, 1024], dt.float32, tag="mmW")
        for j4 in range(4):
            j = half * 4 + j4
            em.mm(psv[:, j4 * H:(j4 + 1) * H], featsf[j], wv_name, wt=wt)
        nc.scalar.copy(vsb[:, half * 4 * H:(half + 1) * 4 * H],
                       psv[:, :4 * H])
    vv = vsb.rearrange("p (j a d) -> p a d j", j=NN, a=HEADS)
    av = attn_n.unsqueeze(2).broadcast_to([128, HEADS, DH, NN])
    t2 = em.p_big.tile([128, HEADS, DH, NN], dt.bfloat16, tag="kvsb")
    nc.gpsimd.tensor_tensor(out=t2, in0=vv, in1=av, op=ALU.mult)
    m = em.p_act1.tile([128, H], dt.float32, tag="mixm")
    nc.vector.tensor_reduce(out=m.rearrange("p (a d) -> p a d", a=HEADS),
                            in_=t2, axis=mybir.AxisListType.X, op=ALU.add)
    if not _zero(em.arrays[bv_name]):
        nm = bv_name + "_repl"
        if nm not in em.arrays:
            em.arrays[nm] = _repl(em.arrays[bv_name].astype(F32)[0, :H])
        m2 = em.p_big.tile([128, H], dt.float32, tag="mixm2")
        nc.vector.tensor_tensor(out=m2, in0=m, in1=em.stream_weight(nm, "s_bvr"),
                                op=ALU.add)
        m = m2
    m_bf = em.p_big.tile([128, H], dt.bfloat16, tag="mixmb")
    nc.gpsimd.tensor_copy(out=m_bf, in_=m)
    return m_bf


# ----------------------------------------------------------------------------
# Stages
# ----------------------------------------------------------------------------

def _emit_round(em, feats, rnd, s):
    nc = em.nc
    featsf = [em.to_f(feats[n], f"ff{n}{s}", pool=em.p_feat) for n in range(NN)]

    qt = em.p_big.tile([128, NN * H], dt.bfloat16, tag="qtr")
    s_all = em.p_big.tile([128, NN, HEADS, NN], dt.float32, tag="sall")
    for n in range(NN):
        wqk = em.stage_weight(f'wqk_{n}')
        psq = em.p_psS.tile([128, 512], dt.float32, tag="mmS")
        em.mm(psq[:, :H], featsf[n], f'wq_{n}', bias=f'bq_{n}',
              wt=wqk[:, :2 * H])
        nc.vector.tensor_copy(out=qt[:, n * H:(n + 1) * H], in_=psq[:, :H])
        _node_scores(em, featsf, qt, n, s_all, wqk[:, 2 * H:], f'wk_{n}')
    attn = _softmax(em, s_all, NN)

    gdot = em.p_stat.tile([128, NN], dt.float32, tag="gdot")
    s0 = em.p_stat.tile([128, NN], dt.float32, tag="ln0_s")
    q0 = em.p_stat.tile([128, NN], dt.float32, tag="ln0_q")
    sg = em.p_stat.tile([128, NN], dt.float32, tag="gate_s")
    qg = em.p_stat.tile([128, NN], dt.float32, tag="gate_q")
    gz_list, o_list = [], []
    for n in range(NN):
        wvo = em.stream_weight(f'wvo_{n}', 's_wvo')
        m_bf = _mix(em, featsf, attn[:, n], f'wv_{n}', f'bv_{n}', "r",
                    wt=wvo[:, :2 * H])
        mf = em.to_f(m_bf, "mf")
        pso = em.p_psS.tile([128, 512], dt.float32, tag="mmS")
        em.mm(pso[:, :H], mf, f'wo_{n}', bias=f'bo_{n}', wt=wvo[:, 2 * H:])
        o_sb = em.p_feat.tile([128, H], dt.bfloat16, tag=f"o{n}{s}")
        nc.vector.tensor_copy(out=o_sb, in_=pso[:, :H])
        o_list.append(o_sb)

        of = em.to_f(o_sb, "of")
        psg = em.p_psS.tile([128, 512], dt.float32, tag="mmS")
        zg = psg[:, :H + 1]
        _, _, neff = em.arrays[f'wg1_{n}']
        wt = em.stage_weight(f'wg1_{n}')
        first = True
        if not _zero(em.arrays[f'bg1_{n}']):
            nc.tensor.matmul(zg, lhsT=em.ones_row,
                             rhs=em.stage_weight(f'bg1_{n}'),
                             start=True, stop=False)
            first = False
        for c in range(4):
            src = featsf[n] if c < 2 else of
            cc = c % 2
            nc.tensor.matmul(zg, lhsT=src[:, cc * 128:(cc + 1) * 128],
                             rhs=wt[:, c * neff:(c + 1) * neff],
                             start=first, stop=(c == 3))
            first = False
        em.ln_stats(zg, H, sg[:, n:n + 1], qg[:, n:n + 1])
        gz = em.p_feat.tile([128, H], dt.bfloat16, tag=f"gz{n}{s}")
        nc.vector.tensor_copy(out=gz, in_=zg[:, :H])
        gz_list.append(gz)

    rstd, nmr = em.ln_finalize(sg, qg, H, NN)
    for n in range(NN):
        gzb = gz_list[n]
        em.ln_apply(gzb, gz_list[n], rstd[:, n:n + 1], nmr[:, n:n + 1],
                    gelu=True, gb_name=f'gg1_{n}')
        dscr = em.p_scr.tile([128, H], dt.bfloat16, tag="dot")
        nc.vector.scalar_tensor_tensor(out=dscr, in0=gzb, scalar=1.0,
                                       in1=em.stage_weight(f'wg2_{n}'),
                                       op0=ALU.bypass, op1=ALU.mult,
                                       accum_out=gdot[:, n:n + 1])
    gate = em.p_stat.tile([128, NN], dt.float32, tag="gate")
    if _zero(em.arrays['bg2']):
        nc.scalar.activation(gate, gdot, AF.Sigmoid)
    else:
        gsum = em.p_stat.tile([128, NN], dt.float32, tag="gatesum")
        nc.vector.tensor_tensor(out=gsum, in0=gdot,
                                in1=em.stage_weight('bg2'), op=ALU.add)
        nc.scalar.activation(gate, gsum, AF.Sigmoid)

    xs = []
    for n in range(NN):
        x = em.p_feat.tile([128, H], dt.float32, tag=f"x{n}{s}")
        nc.vector.scalar_tensor_tensor(out=x, in0=o_list[n],
                                       scalar=gate[:, n:n + 1], in1=feats[n],
                                       op0=ALU.mult, op1=ALU.add,
                                       accum_out=s0[:, n:n + 1])
        sq2 = em.sqscr(H)
        nc.scalar.activation(sq2, x, AF.Square, accum_out=q0[:, n:n + 1])
        xs.append(x)
    rstd, nmr = em.ln_finalize(s0, q0, H, NN)
    new_feats = []
    for n in range(NN):
        fn = em.p_feat.tile([128, H], dt.bfloat16, tag=f"nf{rnd % 2}_{n}{s}")
        nc.vector.tensor_scalar(out=fn, in0=xs[n], scalar1=rstd[:, n:n + 1],
                                scalar2=nmr[:, n:n + 1], op0=ALU.mult,
                                op1=ALU.add)
        new_feats.append(fn)
    return new_feats


def _emit_global(em, feats, s):
    nc = em.nc
    featsf = [em.to_f(feats[n], f"ff{n}{s}", pool=em.p_feat) for n in range(NN)]
    acc = em.p_act1.tile([128, H], dt.float32, tag=f"rs_t{s}")
    nc.vector.tensor_tensor(out=acc, in0=feats[0], in1=feats[1], op=ALU.add)
    for n in range(2, NN):
        nc.vector.tensor_tensor(out=acc, in0=acc, in1=feats[n], op=ALU.add)
    qg = em.p_act.tile([128, H], dt.bfloat16, tag="qg")
    nc.vector.tensor_scalar(out=qg, in0=acc, scalar1=1.0 / NN, scalar2=None,
                            op0=ALU.mult)
    qgf = em.to_f(qg, "qgf")
    psq = em.p_psS.tile([128, 512], dt.float32, tag="mmS")
    em.mm(psq[:, :H], qgf, 'wgq', bias='bgq')
    qt = em.p_act.tile([128, H], dt.bfloat16, tag="gqt")
    nc.vector.tensor_copy(out=qt, in_=psq[:, :H])

    s_all = em.p_big.tile([128, 1, HEADS, NN], dt.float32, tag="gsall")
    _node_scores(em, featsf, qt, 0, s_all, None, 'wgk')
    attn = _softmax(em, s_all, 1)
    m_bf = _mix(em, featsf, attn[:, 0], 'wgv', 'bgv', "g")

    mf = em.to_f(m_bf, "gmf")
    pso = em.p_psS.tile([128, 512], dt.float32, tag="mmS")
    em.mm(pso[:, :H + 1], mf, 'wgo', bias='bgo')
    state = em.p_act.tile([128, H], dt.float32, tag=f"state{s}")
    nc.vector.tensor_copy(out=state, in_=pso[:, :H])
    st_sum = em.p_stat.tile([128, 1], dt.float32, tag=f"stsum{s}")
    nc.vector.tensor_copy(out=st_sum, in_=pso[:, H:H + 1])
    return state, st_sum


def _emit_reasoning(em, state, st_sum, l, s, rsw=None):
    nc = em.nc
    ssq = em.p_stat.tile([128, 1], dt.float32, tag="rs_q1")
    sq3 = em.sqscr(H)
    nc.scalar.activation(sq3, state, AF.Square, accum_out=ssq)
    rstd, nmr = em.ln_finalize(st_sum, ssq, H, 1)
    h1 = em.p_act.tile([128, H], dt.bfloat16, tag="rs_h1")
    em.ln_apply(h1, state, rstd[:, 0:1], nmr[:, 0:1], gelu=False,
                gb_name=f'rs_g1_{l}')
    if rsw is None:
        rsw = em.stream_weight(f'rs_{l}', 's_rs', bufs1=True)
    w_vo_w = em.arrays[f'rs_wvo_{l}'][1] * em.arrays[f'rs_wvo_{l}'][2]
    w_f1_w = 2 * em.arrays[f'rs_wf1a_{l}'][1] * em.arrays[f'rs_wf1a_{l}'][2]
    h1f = em.to_f(h1, "rs_h1f")
    psa = em.p_psS.tile([128, 512], dt.float32, tag="mmS")
    em.mm(psa[:, :H], h1f, f'rs_wvo_{l}', bias=f'rs_bvo_{l}',
          wt=rsw[:, :w_vo_w])
    s1 = em.p_act1.tile([128, H], dt.float32, tag=f"rs_s1{s}")
    s1_sum = em.p_stat.tile([128, 1], dt.float32, tag="rs_s1s")
    nc.vector.scalar_tensor_tensor(out=s1, in0=psa[:, :H], scalar=1.0,
                                   in1=state, op0=ALU.bypass, op1=ALU.add,
                                   accum_out=s1_sum)
    ssq2 = em.p_stat.tile([128, 1], dt.float32, tag="rs_q2")
    sq4 = em.sqscr(H)
    nc.scalar.activation(sq4, s1, AF.Square, accum_out=ssq2)
    rstd, nmr = em.ln_finalize(s1_sum, ssq2, H, 1)
    h2 = em.p_act.tile([128, H], dt.bfloat16, tag="rs_h2")
    em.ln_apply(h2, s1, rstd[:, 0:1], nmr[:, 0:1], gelu=False,
                gb_name=f'rs_g2_{l}')
    h2f = em.to_f(h2, "rs_h2f")
    ff = em.p_act1.tile([128, 4 * H], dt.bfloat16, tag="rs_ff")
    wfa = em.arrays[f'rs_wf1a_{l}'][1] * em.arrays[f'rs_wf1a_{l}'][2]
    psf = em.p_psW.tile([128, 1024], dt.float32, tag="mmW")
    for hh in range(2):
        em.mm(psf[:, hh * 512:(hh + 1) * 512], h2f,
              f'rs_wf1a_{l}' if hh == 0 else f'rs_wf1b_{l}',
              bias=f'rs_bf1a_{l}' if hh == 0 else f'rs_bf1b_{l}',
              wt=rsw[:, w_vo_w + hh * wfa:w_vo_w + (hh + 1) * wfa])
    nc.scalar.activation(ff, psf, AF.Gelu)
    fff = em.to_f(ff, "rs_fff", pool=em.p_act1)
    psf2 = em.p_psS.tile([128, 512], dt.float32, tag="mmS")
    em.mm(psf2[:, :H], fff, f'rs_wf2_{l}', bias=f'rs_bf2_{l}',
          wt=rsw[:, w_vo_w + w_f1_w:])
    t = em.p_act1.tile([128, H], dt.float32, tag=f"rs_t{s}")
    nc.vector.scalar_tensor_tensor(out=t, in0=psf2[:, :H], scalar=1.0, in1=s1,
                                   op0=ALU.bypass, op1=ALU.add)
    state2 = em.p_act.tile([128, H], dt.float32, tag=f"state{s}")
    st_sum2 = em.p_stat.tile([128, 1], dt.float32, tag=f"stsum{s}")
    nc.vector.scalar_tensor_tensor(out=state2, in0=t, scalar=1.0, in1=state,
                                   op0=ALU.bypass, op1=ALU.add,
                                   accum_out=st_sum2)
    return state2, st_sum2


def _emit_head(em, state, st_sum, y_d, r0, s):
    nc = em.nc
    sb = em.p_act.tile([128, H], dt.bfloat16, tag="hd_sb")
    nc.gpsimd.tensor_copy(out=sb, in_=state)
    sf = em.to_f(sb, "hd_sf")
    ps1 = em.p_psS.tile([128, 512], dt.float32, tag="mmS")
    z1 = ps1[:, :H + 1]
    em.mm(z1, sf, 'wo1', bias='bo1')
    s = em.p_stat.tile([128, 2], dt.float32, tag="hd_s")
    q = em.p_stat.tile([128, 2], dt.float32, tag="hd_q")
    em.ln_stats(z1, H, s[:, 0:1], q[:, 0:1])
    rstd, nmr = em.ln_finalize(s[:, 0:1], q[:, 0:1], H, 1)
    y1 = em.p_act.tile([128, H], dt.bfloat16, tag="hd_y1")
    em.ln_apply(y1, z1[:, :H], rstd[:, 0:1], nmr[:, 0:1], gelu=True,
                gb_name='go1')
    # y2 = y1 @ Wo2.T + bo2  (no LN)
    y1f = em.to_f(y1, "hd_y1f")
    ps2 = em.p_psS.tile([128, 512], dt.float32, tag="mmS")
    em.mm(ps2[:, :H // 2], y1f, 'wo2', bias='bo2')
    y2 = em.p_act.tile([128, H // 2], dt.bfloat16, tag="hd_y2")
    nc.vector.tensor_copy(out=y2, in_=ps2[:, :H // 2])
    # y3 = gelu(LN(y2 @ Wf1.T + bf1))
    y2f = em.to_f(y2, "hd_y2f")
    ps3 = em.p_psS.tile([128, 512], dt.float32, tag="mmS")
    z3 = ps3[:, :H // 2 + 1]
    em.mm(z3, y2f, 'wf1', bias='bf1')
    em.ln_stats(z3, H // 2, s[:, 1:2], q[:, 1:2])
    rstd, nmr = em.ln_finalize(s[:, 1:2], q[:, 1:2], H // 2, 1)
    y3 = em.p_act.tile([128, H // 2], dt.bfloat16, tag="hd_y3")
    em.ln_apply(y3, z3[:, :H // 2], rstd[:, 0:1], nmr[:, 0:1], gelu=True,
                gb_name='gf1')
    # y = y3 . wf2 + bf2
    ydot = em.p_stat.tile([128, 1], dt.float32, tag="ydot")
    dscr = em.p_scr.tile([128, H // 2], dt.bfloat16, tag="dot")
    nc.vector.scalar_tensor_tensor(out=dscr, in0=y3,
                                   scalar=1.0, in1=em.stage_weight('wf2'),
                                   op0=ALU.bypass, op1=ALU.mult,
                                   accum_out=ydot)
    y_sb = em.p_act.tile([128, 1], dt.float32, tag="hd_y")
    nc.scalar.activation(y_sb, ydot, AF.Copy, bias=float(em.arrays['bf2']))
    nc.sync.dma_start(out=y_d[r0:r0 + PT, :], in_=y_sb)


def _stage_load_embed(em, st):
    nc = em.nc
    s = st['s']
    xb = em.p_act1.tile([128, D], dt.bfloat16, tag="xb")
    nc.sync.dma_start(out=xb, in_=st['x_d'][st['r0']:st['r0'] + PT, :])
    xf = em.to_f(xb, "xf")
    ps = em.p_psW.tile([128, 1024], dt.float32, tag="mmW")
    z = ps[:, :2 * H + 1]
    em.mm(z, xf, 'we1', bias='be1')
    s1 = em.p_stat.tile([128, 2], dt.float32, tag="emb_s")
    q1 = em.p_stat.tile([128, 2], dt.float32, tag="emb_q")
    em.ln_stats(z, 2 * H, s1[:, 0:1], q1[:, 0:1])
    rstd, nmr = em.ln_finalize(s1[:, 0:1], q1[:, 0:1], 2 * H, 1)
    h1 = em.p_act.tile([128, 2 * H], dt.bfloat16, tag="h1")
    em.ln_apply(h1, z[:, :2 * H], rstd[:, 0:1], nmr[:, 0:1], gelu=True,
                gb_name='ge1')

    h1f = em.to_f(h1, "h1f")
    ps2 = em.p_psS.tile([128, 512], dt.float32, tag="mmS")
    z2 = ps2[:, :H + 1]
    em.mm(z2, h1f, 'we2', bias='be2')
    em.ln_stats(z2, H, s1[:, 1:2], q1[:, 1:2])
    rstd, nmr = em.ln_finalize(s1[:, 1:2], q1[:, 1:2], H, 1)
    h = em.p_act.tile([128, H], dt.bfloat16, tag="h")
    em.ln_apply(h, z2[:, :H], rstd[:, 0:1], nmr[:, 0:1], gelu=True,
                gb_name='ge2')
    st['h'] = h


def _stage_nodes(em, st):
    nc = em.nc
    s2 = st['s']
    hf = em.to_f(st['h'], "hf")
    feats = []
    NG = 2
    for grp in range(NN // NG):
        zs = []
        wns = []
        sgr = em.p_stat.tile([128, NG], dt.float32, tag="nd_s")
        qgr = em.p_stat.tile([128, NG], dt.float32, tag="nd_q")
        for i in range(NG):
            n = grp * NG + i
            psn = em.p_psW.tile([128, 1024], dt.float32, tag="mmW")
            zn = psn[:, :2 * H + 1]
            wn = em.stream_weight(f'wn_{n}', 's_wn')
            wns.append(wn)
            w1w = em.arrays[f'wn1_{n}'][1] * em.arrays[f'wn1_{n}'][2]
            em.mm(zn, hf, f'wn1_{n}', bias=f'bn1_{n}', wt=wn[:, :w1w])
            em.ln_stats(zn, 2 * H, sgr[:, i:i + 1], qgr[:, i:i + 1])
            zs.append(zn)
        rstd, nmr = em.ln_finalize(sgr, qgr, 2 * H, NG)
        z1s = []
        for i in range(NG):
            n = grp * NG + i
            z1n = em.p_act.tile([128, 2 * H], dt.bfloat16, tag=f"z1_{i}")
            em.ln_apply(z1n, zs[i][:, :2 * H], rstd[:, i:i + 1],
                        nmr[:, i:i + 1], gelu=True, gb_name=f'gn1_{n}')
            z1s.append(z1n)
        sg2 = em.p_stat.tile([128, NG], dt.float32, tag="nd_s2")
        qg2 = em.p_stat.tile([128, NG], dt.float32, tag="nd_q2")
        zps = []
        for i in range(NG):
            n = grp * NG + i
            z1f = em.to_f(z1s[i], f"z1f_{i}")
            psn = em.p_psS.tile([128, 512], dt.float32, tag="mmS")
            zn = psn[:, :H + 1]
            w1w = em.arrays[f'wn1_{n}'][1] * em.arrays[f'wn1_{n}'][2]
            em.mm(zn, z1f, f'wn2_{n}', bias=f'bn2_{n}', wt=wns[i][:, w1w:])
            em.ln_stats(zn, H, sg2[:, i:i + 1], qg2[:, i:i + 1])
            zps.append(zn)
        rstd, nmr = em.ln_finalize(sg2, qg2, H, NG)
        for i in range(NG):
            n = grp * NG + i
            fn = em.p_feat.tile([128, H], dt.bfloat16, tag=f"nf1_{n}{s2}")
            em.ln_apply(fn, zps[i][:, :H], rstd[:, i:i + 1], nmr[:, i:i + 1],
                        gelu=True, gb_name=f'gn2_{n}')
            feats.append(fn)
    st['feats'] = feats


def _stage_round(em, st, rnd):
    st['feats'] = _emit_round(em, st['feats'], rnd, st['s'])


def _stage_global(em, st):
    state, st_sum = _emit_global(em, st['feats'], st['s'])
    st['state'] = state
    st['st_sum'] = st_sum


def _stage_reasoning(em, st, l, rsw=None):
    state, st_sum = _emit_reasoning(em, st['state'], st['st_sum'], l, st['s'],
                                    rsw=rsw)
    st['state'] = state
    st['st_sum'] = st_sum


def _stage_head(em, st):
    _emit_head(em, st['state'], st['st_sum'], st['y_d'], st['r0'], st['s'])


# ----------------------------------------------------------------------------
# Program build + run
# ----------------------------------------------------------------------------

def build_program(arrays, b_core=BC):
    from contextlib import ExitStack
    nc = bacc.Bacc("TRN2", target_bir_lowering=False, debug=False)
    ntiles = b_core // PT
    with tile.TileContext(nc) as tc:
        with ExitStack() as ctx:
            em = Emitter(nc, tc, ctx, arrays)
            x_d = em.add_input("x", np.zeros((b_core, D), BF16))
            y_h = nc.dram_tensor("y", [b_core, 1], dt.float32,
                                 kind="ExternalOutput")
            y_d = y_h.ap()
            GS = 2
            for t0 in range(0, ntiles, GS):
                pair = [dict(s=si, x_d=x_d, y_d=y_d, r0=(t0 + si) * PT)
                        for si in range(min(GS, ntiles - t0))]
                for st in pair:
                    _stage_load_embed(em, st)
                for st in pair:
                    _stage_nodes(em, st)
                for rnd in range(3):
                    for st in pair:
                        _stage_round(em, st, rnd)
                for st in pair:
                    _stage_global(em, st)
                for l in range(L):
                    rsw = em.stream_weight(f'rs_{l}', 's_rs', bufs1=True)
                    for st in pair:
                        _stage_reasoning(em, st, l, rsw)
                for st in pair:
                    _stage_head(em, st)
    nc.compile()
    # input name -> host array (weights); 'x' filled per core at run time
    wmap = {}
    for name, ap in em.dram.items():
        if name == 'x':
            continue
        arr = arrays[name[2:]]
        if isinstance(arr, tuple):
            arr = arr[0]
        wmap[name] = np.ascontiguousarray(arr)
    return nc, wmap


_CACHE = {}


def kernel(x, params):
    x = np.asarray(x, F32)
    assert x.shape == (B, D), x.shape
    arrays = _prep(params)
    key = "prog"
    if key not in _CACHE:
        # Note: program structure (zero-bias / identity-affine gates) is
        # specialized to the first call's params; weight VALUES are re-read
        # from `arrays` on every call below.
        _CACHE[key] = build_program(arrays, BC)
    nc, wmap = _CACHE[key]
    fresh = {}
    for name in wmap:
        arr = arrays.get(name[2:])
        if arr is None:
            fresh[name] = wmap[name]  # derived array created at build time
            continue
        if isinstance(arr, tuple):
            arr = arr[0]
        fresh[name] = np.ascontiguousarray(arr)
    in_maps = []
    for c in range(NCORES):
        m = dict(fresh)
        m['x'] = np.ascontiguousarray(x[c * BC:(c + 1) * BC]).astype(BF16)
        in_maps.append(m)
    res = bass_utils.run_bass_kernel_spmd(nc, in_maps,
                                          core_ids=list(range(NCORES)))
    out = np.concatenate([res.results[c]['y'] for c in range(NCORES)], axis=0)
    return out.astype(F32)
